# revision 1
# baseline (speedup 1.0000x reference)
"""GAT kernel for TRN2: host prep + Bass program builder + numpy model.

Sharding: nodes (and their in-edges) partitioned across cores by contiguous
shard; per dst-block-of-128 selector-matmul scatter; edge gathers of packed
table rows [h(128f32) | al(4f32) | pad] = 768B via gpsimd dma_gather with a
lo/hi table split (int16 index limit); inter-layer AllGather of the table;
BN via AllReduce of partial sums; pooling via transposed graph-selector
matmul; tiny FC + final AllReduce.
"""
from dataclasses import dataclass

import numpy as np

import concourse.bacc as bacc
import concourse.bass as bass
import concourse.mybir as mybir
import concourse.tile as tile
from concourse import library_config

F32 = mybir.dt.float32
I16 = mybir.dt.int16
I8 = mybir.dt.int8
AX = mybir.AluOpType
AF = mybir.ActivationFunctionType


class _SkipRest(Exception):
    pass



@dataclass
class Cfg:
    ncores: int = 8
    n_real: int = 50000       # real nodes
    np_: int = 50176          # padded nodes (multiple of ncores*128)
    e_raw: int = 800000       # edges before self loops
    g: int = 500              # graphs
    gp: int = 512             # padded graphs (pool matmul free dim)
    tlo: int = 0              # max tiles per block from lo table (computed in prep)
    thi: int = 0
    tlo_b: tuple = ()         # per-block lo tile counts (max over cores)
    thi_b: tuple = ()
    f: int = 128              # features (in = out = 128)
    h: int = 4
    c: int = 32
    k: int = 6
    eps: float = 1e-5
    rowf: int = 192           # table row floats (768B); 128 when bf16_rows
    stage: int = 9            # debug: how much of the program to emit
    repeat: int = 1           # timing: execute the whole body N times
    bf16_rows: bool = False   # pack h as bf16 in 512B table rows
    bf16_mm: bool = False     # bf16 selectors + weighted features (FWL matmuls)
    neg_slope: float = 0.2

    @property
    def shard(self):
        return self.np_ // self.ncores

    @property
    def nblk(self):
        return self.shard // 128

    @property
    def half(self):
        return self.np_ // 2

    @property
    def tpb(self):
        return self.tlo + self.thi


def fold_attn(a, H, C):
    A = np.zeros((H * C, H), np.float32)
    for h in range(H):
        A[h * C:(h + 1) * C, h] = a[h]
    return A


def pack_idx16(idx):
    """int array [n] (n % 128 == 0) -> [128, n//16] int16 dma_gather layout."""
    n = len(idx)
    arr = np.zeros((16, n // 16), dtype=np.int16)
    arr[np.arange(n) % 16, np.arange(n) // 16] = idx
    return np.tile(arr, (8, 1))


def prep_edges(cfg: Cfg, edge_index):
    """Returns srcp/dstp [ncores, nblk, tpb, 128] (int64; dst sentinel 999)
    and sets cfg.tlo/thi."""
    n, npd, sh = cfg.n_real, cfg.np_, cfg.shard
    src = np.concatenate([edge_index[0], np.arange(n)]).astype(np.int64)
    dst = np.concatenate([edge_index[1], np.arange(n)]).astype(np.int64)
    order = np.argsort(dst // sh, kind="stable")
    buckets = [[None] * cfg.nblk for _ in range(cfg.ncores)]
    tlo_b = [1] * cfg.nblk
    thi_b = [1] * cfg.nblk
    for ci in range(cfg.ncores):
        m = (dst // sh) == ci
        s, d = src[m], dst[m] - ci * sh
        for b in range(cfg.nblk):
            mb = (d // 128) == b
            sb, db = s[mb], d[mb] % 128
            lo = sb < cfg.half
            buckets[ci][b] = ((sb[lo], db[lo]), (sb[~lo] - cfg.half, db[~lo]))
            tlo_b[b] = max(tlo_b[b], -(-len(sb[lo]) // 128))
            thi_b[b] = max(thi_b[b], -(-int((~lo).sum()) // 128))
    cfg.tlo, cfg.thi = max(tlo_b), max(thi_b)
    cfg.tlo_b, cfg.thi_b = tuple(tlo_b), tuple(thi_b)
    tlo, thi = cfg.tlo, cfg.thi
    tpb = cfg.tpb
    srcp = np.zeros((cfg.ncores, cfg.nblk, tpb, 128), np.int64)
    dstp = np.full((cfg.ncores, cfg.nblk, tpb, 128), 999, np.int64)
    for ci in range(cfg.ncores):
        for b in range(cfg.nblk):
            (slo, dlo), (shi, dhi) = buckets[ci][b]
            srcp[ci, b, :tlo].flat[: len(slo)] = slo
            dstp[ci, b, :tlo].flat[: len(dlo)] = dlo
            srcp[ci, b, tlo:].flat[: len(shi)] = shi
            dstp[ci, b, tlo:].flat[: len(dhi)] = dhi
    return srcp, dstp


def prep_inputs(cfg: Cfg, inputs):
    """Build per-core in_maps (list of dicts)."""
    H, C, F = cfg.h, cfg.c, cfg.f
    srcp, dstp = prep_edges(cfg, inputs["edge_index"])
    tlo, thi, tpb = cfg.tlo, cfg.thi, cfg.tpb

    xpad = np.zeros((cfg.np_, F), np.float32)
    xpad[: cfg.n_real] = inputs["x"]

    W1e = np.concatenate(
        [inputs["W1"], inputs["W1"] @ fold_attn(inputs["a_src1"], H, C),
         inputs["W1"] @ fold_attn(inputs["a_dst1"], H, C)], axis=1)  # [F,136]
    W2e = np.concatenate(
        [inputs["W2"], inputs["W2"] @ fold_attn(inputs["a_src2"], H, C),
         inputs["W2"] @ fold_attn(inputs["a_dst2"], H, C)], axis=1)

    batch = np.asarray(inputs["batch"]).astype(np.int64)
    batch_pad = np.full(cfg.np_, 999, np.int64)
    batch_pad[: cfg.n_real] = batch
    cnt = np.bincount(batch, minlength=cfg.gp).astype(np.float32)
    rcnt = (1.0 / np.maximum(cnt, 1.0)).astype(np.float32)

    shared = dict(
        w1e=W1e.astype(np.float32), w2e=W2e.astype(np.float32),
        b1bc=np.tile(inputs["b1"][None, :], (128, 1)).astype(np.float32),
        b2bc=np.tile(inputs["b2"][None, :], (128, 1)).astype(np.float32),
        g1row=inputs["g1"][None, :].astype(np.float32),
        be1row=inputs["be1"][None, :].astype(np.float32),
        g2row=inputs["g2"][None, :].astype(np.float32),
        be2row=inputs["be2"][None, :].astype(np.float32),
        fcw=inputs["fcW"].astype(np.float32),
        fcbbc=np.tile(inputs["fcb"][:, None], (1, cfg.gp)).astype(np.float32),
        rcntbc=np.tile(rcnt[None, :], (cfg.k, 1)).astype(np.float32),
        ident=np.eye(128, dtype=np.float32),
        diota=np.tile(np.arange(128, dtype=np.float32)[None, :], (128, 1)).astype(
            mybir.dt.np(mybir.dt.bfloat16) if cfg.bf16_mm else np.float32),
        piota=np.arange(128, dtype=np.float32)[:, None].copy(),
        piota8=np.arange(128, dtype=np.int8)[:, None].copy(),
        giota=np.tile(np.arange(cfg.gp, dtype=np.float32)[None, :], (128, 1)),
        onescol=np.ones((128, 1), np.float32),
        onesrow=np.ones((1, 128), np.float32),
    )

    in_maps = []
    for ci in range(cfg.ncores):
        idx_lo = np.zeros((cfg.nblk, 128, tlo * 8), np.int16)
        idx_hi = np.zeros((cfg.nblk, 128, thi * 8), np.int16)
        for b in range(cfg.nblk):
            idx_lo[b] = pack_idx16(srcp[ci, b, :tlo].reshape(-1))
            idx_hi[b] = pack_idx16(srcp[ci, b, tlo:].reshape(-1))
        dflat = dstp[ci].reshape(cfg.nblk, tpb * 128)
        drow8 = np.where(dflat > 127, -1, dflat).astype(np.int8)
        drow8 = np.broadcast_to(drow8[:, None, :], (cfg.nblk, 128, tpb * 128)).copy()
        dstsel = dstp[ci].transpose(0, 2, 1)  # [nblk, tpb, 128] -> want [nblk,128,tpb]
        seldt = mybir.dt.np(mybir.dt.bfloat16) if cfg.bf16_mm else np.float32
        # [128, nblk, tpb]: partition-major so the SBUF load is 128 big descriptors
        dstsel = np.ascontiguousarray(
            np.transpose(dstp[ci], (2, 0, 1))).astype(seldt)
        sl = slice(ci * cfg.shard, (ci + 1) * cfg.shard)
        nm = np.zeros((128, cfg.nblk), np.float32)
        bc = np.zeros((128, cfg.nblk), np.float32)
        ids = np.arange(ci * cfg.shard, (ci + 1) * cfg.shard)
        nm[:] = (ids.reshape(cfg.nblk, 128).T < cfg.n_real)
        bc[:] = batch_pad[ids].reshape(cfg.nblk, 128).T.astype(np.float32)
        xs = xpad[sl].reshape(cfg.nblk, 128, F).transpose(1, 0, 2)
        in_maps.append(dict(
            x_shard=np.ascontiguousarray(xs).reshape(128, cfg.nblk * F),
            idx_lo=idx_lo, idx_hi=idx_hi,
            dstsel=dstsel,
            dstrow=drow8,
            node_mask=nm, batchcol=bc,
            **shared,
        ))
    return in_maps


# ---------------------------------------------------------------------------
# numpy model (for validation at any cfg)
# ---------------------------------------------------------------------------

def numpy_forward(cfg: Cfg, inputs):
    H, C, F = cfg.h, cfg.c, cfg.f
    srcp, dstp = prep_edges(cfg, inputs["edge_index"])
    xpad = np.zeros((cfg.np_, F), np.float32)
    xpad[: cfg.n_real] = inputs["x"]

    def layer(xp, W, asrc, adst, b):
        We = np.concatenate([W, W @ fold_attn(asrc, H, C), W @ fold_attn(adst, H, C)], 1)
        tab = xp @ We
        out = np.zeros((cfg.np_, F), np.float32)
        for ci in range(cfg.ncores):
            for bi in range(cfg.nblk):
                base = ci * cfg.shard + bi * 128
                acc = np.zeros((128, F + H), np.float32)
                ar_blk = tab[base: base + 128, F + H: F + 2 * H]
                for t in range(cfg.tpb):
                    s = srcp[ci, bi, t] + (cfg.half if t >= cfg.tlo else 0)
                    dl = dstp[ci, bi, t]
                    grow = tab[s]
                    sel = (dl[:, None] == np.arange(128)[None, :]).astype(np.float32)
                    e = grow[:, F:F + H] + sel @ ar_blk
                    e = np.where(e > 0, e, cfg.neg_slope * e).astype(np.float32)
                    p = np.exp(e).astype(np.float32)
                    w = grow[:, :F] * np.repeat(p, C, 1)
                    acc += sel.T @ np.concatenate([w, p], 1)
                ssum = np.maximum(np.repeat(acc[:, F:], C, 1), 1e-30)
                out[base:base + 128] = acc[:, :F] / ssum + b
        return out

    def bn_elu(hh, g, be):
        s, ss = hh[:cfg.n_real].sum(0), (hh[:cfg.n_real] ** 2).sum(0)
        mu = s / cfg.n_real
        var = ss / cfg.n_real - mu ** 2
        sc = g / np.sqrt(var + cfg.eps)
        sh = be - mu * sc
        y = hh * sc + sh
        return (np.where(y > 0, y, np.exp(np.minimum(y, 0)) - 1)).astype(np.float32)

    h1 = layer(xpad, inputs["W1"], inputs["a_src1"], inputs["a_dst1"], inputs["b1"])
    h1n = bn_elu(h1, inputs["g1"], inputs["be1"])
    h2 = layer(h1n, inputs["W2"], inputs["a_src2"], inputs["a_dst2"], inputs["b2"])
    h2n = bn_elu(h2, inputs["g2"], inputs["be2"])

    batch = np.asarray(inputs["batch"]).astype(np.int64)
    gsel = np.zeros((cfg.n_real, cfg.gp), np.float32)
    gsel[np.arange(cfg.n_real), batch] = 1.0
    pooled = h2n[:cfg.n_real].T @ gsel
    fc = inputs["fcW"].T.astype(np.float32) @ pooled
    cnt = np.bincount(batch, minlength=cfg.gp).astype(np.float32)
    fc = fc / np.maximum(cnt, 1.0)[None, :] + inputs["fcb"][:, None]
    return fc[:, :cfg.g].T  # [g, k]


# ---------------------------------------------------------------------------
# Bass program
# ---------------------------------------------------------------------------

def build_nc(cfg: Cfg):
    NB, TPB, TLO, THI = cfg.nblk, cfg.tpb, cfg.tlo, cfg.thi
    F, H, C, RF = cfg.f, cfg.h, cfg.c, cfg.rowf
    FH = F + H
    SH = cfg.shard
    GP = cfg.gp

    if cfg.bf16_mm:
        assert cfg.bf16_rows, "bf16_mm requires bf16_rows"
    if cfg.bf16_rows:
        cfg.rowf = 128
        RF = 128
    ALO = 64 if cfg.bf16_rows else F   # f32-slot offset of al in a row
    BF = mybir.dt.bfloat16
    MMDT = BF if cfg.bf16_mm else F32
    nc = bacc.Bacc("TRN2", target_bir_lowering=False, debug=False,
                   num_devices=cfg.ncores, num_swdge_queues=4)

    def ext(name, shape, dtype=F32):
        return nc.dram_tensor(name, shape, dtype, kind="ExternalInput")

    x_shard = ext("x_shard", [128, NB * F])
    idx_lo = ext("idx_lo", [NB, 128, TLO * 8], I16)
    idx_hi = ext("idx_hi", [NB, 128, THI * 8], I16)
    dstsel_d = ext("dstsel", [128, NB, TPB], MMDT)
    dstrow_d = ext("dstrow", [NB, 128, TPB * 128], I8)
    node_mask = ext("node_mask", [128, NB])
    batchcol = ext("batchcol", [128, NB])
    w1e = ext("w1e", [F, F + 2 * H])
    w2e = ext("w2e", [F, F + 2 * H])
    b1bc = ext("b1bc", [128, F])
    b2bc = ext("b2bc", [128, F])
    g1row = ext("g1row", [1, F])
    be1row = ext("be1row", [1, F])
    g2row = ext("g2row", [1, F])
    be2row = ext("be2row", [1, F])
    fcw = ext("fcw", [F, cfg.k])
    fcbbc = ext("fcbbc", [cfg.k, GP])
    rcntbc = ext("rcntbc", [cfg.k, GP])
    ident_d = ext("ident", [128, 128])
    diota_d = ext("diota", [128, 128], MMDT)
    piota_d = ext("piota", [128, 1])
    piota8_d = ext("piota8", [128, 1], I8)
    giota_d = ext("giota", [128, GP])
    onescol_d = ext("onescol", [128, 1])
    onesrow_d = ext("onesrow", [1, 128])

    out_d = nc.dram_tensor("out", [cfg.k, GP], F32, kind="ExternalOutput")

    rg = [list(range(cfg.ncores))]
    shared_as = "Shared" if cfg.ncores > 4 else "Local"

    with tile.TileContext(nc) as tc:
        with (
            tc.tile_pool(name="dram", bufs=1, space="DRAM") as dpool,
            tc.tile_pool(name="persist", bufs=1) as pp,
            tc.tile_pool(name="consts", bufs=1) as cp,
            tc.tile_pool(name="work", bufs=3) as wp_pool,
            tc.tile_pool(name="gath", bufs=3) as gp_pool,
            tc.tile_pool(name="psum", bufs=3, space="PSUM") as ps_pool,
            tc.tile_pool(name="psum1", bufs=1, space="PSUM") as ps1_pool,
        ):
            nc.gpsimd.load_library(library_config.mlp)

            # ---- persistent SBUF ----
            h_cur = pp.tile([128, NB, F], F32)          # shard activations
            ar_sb = pp.tile([128, NB, H], MMDT)
            dstsel_sb = pp.tile([128, NB, TPB], MMDT)
            mask_sb = pp.tile([128, NB], F32)
            bcol_sb = pp.tile([128, NB], F32)

            # ---- constants ----
            w1e_sb = cp.tile([128, F + 2 * H], F32)
            w2e_sb = cp.tile([128, F + 2 * H], F32)
            b1bc_sb = cp.tile([128, F], F32)
            b2bc_sb = cp.tile([128, F], F32)
            ident = cp.tile([128, 128], F32)
            diota = cp.tile([128, 128], MMDT)
            piota = cp.tile([128, 1], F32)
            piota8 = cp.tile([128, 1], I8)
            giota = cp.tile([128, GP], F32)
            onescol = cp.tile([128, 1], F32)
            onesrow = cp.tile([1, 128], F32)
            g1_sb = cp.tile([1, F], F32)
            be1_sb = cp.tile([1, F], F32)
            g2_sb = cp.tile([1, F], F32)
            be2_sb = cp.tile([1, F], F32)
            fcw_sb = cp.tile([128, cfg.k], F32)
            fcbbc_sb = cp.tile([cfg.k, GP], F32)
            rcnt_sb = cp.tile([cfg.k, GP], F32)

            for sb, d in [(w1e_sb, w1e), (w2e_sb, w2e), (b1bc_sb, b1bc),
                          (b2bc_sb, b2bc), (ident, ident_d), (diota, diota_d),
                          (piota, piota_d), (piota8, piota8_d), (giota, giota_d), (onescol, onescol_d),
                          (onesrow, onesrow_d), (g1_sb, g1row), (be1_sb, be1row),
                          (g2_sb, g2row), (be2_sb, be2row), (fcw_sb, fcw),
                          (fcbbc_sb, fcbbc), (rcnt_sb, rcntbc),
                          (dstsel_sb, dstsel_d),
                          (mask_sb, node_mask), (bcol_sb, batchcol),
                          (h_cur, x_shard)]:
                nc.sync.dma_start(sb[:], d[:])

            # ---- DRAM internals ----
            ht_in = [dpool.tile([SH, RF], F32, name=f"ht{i}_in") for i in (1, 2)]
            ht = [dpool.tile([cfg.np_, RF], F32, addr_space=shared_as, name=f"ht{i}")
                  for i in (1, 2)]
            bn_in = [dpool.tile([1, 2 * F], F32, name=f"bn{i}_in") for i in (1, 2)]
            bn_out = [dpool.tile([1, 2 * F], F32, addr_space=shared_as, name=f"bn{i}_out")
                      for i in (1, 2)]
            fc_in = dpool.tile([cfg.k, GP], F32)
            fc_out = dpool.tile([cfg.k, GP], F32, addr_space=shared_as)

            # ================= helper phases =================

            def dense_phase(li, wext_sb):
                """h_cur -> table rows (HT_in) + ar_sb; then AllGather."""
                for b in range(NB):
                    tr_ps = ps_pool.tile([128, 128], F32, tag="psA")
                    nc.tensor.transpose(tr_ps[:], h_cur[:, b, :], ident[:])
                    xT = wp_pool.tile([128, 128], F32, tag="xT")
                    nc.vector.tensor_copy(xT[:], tr_ps[:])
                    dp_ps = ps_pool.tile([128, F + 2 * H], F32, tag="psB")
                    nc.tensor.matmul(dp_ps[:], xT[:], wext_sb[:], start=True, stop=True)
                    row = wp_pool.tile([128, RF], F32, tag="row")
                    if cfg.bf16_rows:
                        nc.vector.tensor_copy(
                            row[:, :64].bitcast(mybir.dt.bfloat16), dp_ps[:, :F])
                        nc.vector.tensor_copy(row[:, 64:64 + H], dp_ps[:, F:FH])
                        nc.vector.memset(row[:, 64 + H:], 0.0)
                    else:
                        nc.vector.tensor_copy(row[:, :FH], dp_ps[:, :FH])
                        nc.vector.memset(row[:, FH:], 0.0)
                    nc.vector.tensor_copy(ar_sb[:, b, :], dp_ps[:, FH:FH + H])
                    nc.sync.dma_start(
                        ht_in[li][b * 128:(b + 1) * 128, :],
                        row[:])
                nc.gpsimd.collective_compute(
                    "AllGather", AX.bypass, replica_groups=rg,
                    ins=[ht_in[li][:]], outs=[ht[li][:]])

            def scatter_phase(li, bbc_sb):
                """edge phase: gathers + selector matmuls -> h_cur (+bias)."""
                tab = ht[li]
                for b in range(NB):
                    TL, TH = cfg.tlo_b[b], cfg.thi_b[b]
                    ilo_t = gp_pool.tile([128, TLO * 8], I16, tag="ilo")
                    ihi_t = gp_pool.tile([128, THI * 8], I16, tag="ihi")
                    nc.sync.dma_start(ilo_t[:, :TL * 8], idx_lo[b, :, :TL * 8])
                    nc.sync.dma_start(ihi_t[:, :TH * 8], idx_hi[b, :, :TH * 8])
                    glo = gp_pool.tile([128, TLO, RF], F32, tag="glo")
                    ghi = gp_pool.tile([128, THI, RF], F32, tag="ghi")
                    nc.gpsimd.dma_gather(
                        out_ap=glo[:, :TL, :], in_ap=tab[:cfg.half, :],
                        idxs_ap=ilo_t[:, :TL * 8],
                        num_idxs=TL * 128, num_idxs_reg=TL * 128, elem_size=RF,
                        queue_num=(b % 2) * 2, single_packet=False)
                    nc.gpsimd.dma_gather(
                        out_ap=ghi[:, :TH, :], in_ap=tab[cfg.half:, :],
                        idxs_ap=ihi_t[:, :TH * 8],
                        num_idxs=TH * 128, num_idxs_reg=TH * 128, elem_size=RF,
                        queue_num=(b % 2) * 2 + 1, single_packet=False)
                    # selectors
                    sel = wp_pool.tile([128, TPB, 128], MMDT, tag="sel")
                    nc.vector.tensor_tensor(
                        sel[:],
                        dstsel_sb[:, b, :].unsqueeze(2).broadcast_to([128, TPB, 128]),
                        diota[:].unsqueeze(1).broadcast_to([128, TPB, 128]),
                        AX.is_equal)
                    drow = gp_pool.tile([128, TPB * 128], I8, tag="drow")
                    nc.sync.dma_start(drow[:], dstrow_d[b, :, :])
                    selT = wp_pool.tile([128, TPB * 128], MMDT, tag="selT")
                    nc.vector.tensor_scalar(selT[:], drow[:], piota[:], None,
                                            AX.is_equal)
                    # ar expand per tile
                    arx_ps = ps_pool.tile([128, TPB, H], F32, tag="psA")
                    real_ts = list(range(TL)) + list(range(TLO, TLO + TH))
                    for t in real_ts:
                        nc.tensor.matmul(arx_ps[:, t, :],
                                         selT[:, t * 128:(t + 1) * 128],
                                         ar_sb[:, b, :], start=True, stop=True)
                    # e = al + ar ; lrelu; exp -> p (into wp[..,128:132])
                    wpt = wp_pool.tile([128, TPB, FH], MMDT, tag="wpt")
                    e_sb = wp_pool.tile([128, TPB, H], F32, tag="e")
                    nc.vector.tensor_tensor(e_sb[:, :TL, :],
                                            glo[:, :TL, ALO:ALO + H],
                                            arx_ps[:, :TL, :], AX.add)
                    nc.vector.tensor_tensor(e_sb[:, TLO:TLO + TH, :],
                                            ghi[:, :TH, ALO:ALO + H],
                                            arx_ps[:, TLO:TLO + TH, :], AX.add)
                    eneg = wp_pool.tile([128, TPB, H], F32, tag="eneg")
                    for r0, r1 in ((0, TL), (TLO, TLO + TH)):
                        nc.vector.tensor_scalar(eneg[:, r0:r1, :], e_sb[:, r0:r1, :],
                                                0.0, cfg.neg_slope, AX.min, AX.mult)
                        nc.vector.tensor_scalar(e_sb[:, r0:r1, :], e_sb[:, r0:r1, :],
                                                0.0, None, AX.max)
                        nc.vector.tensor_tensor(e_sb[:, r0:r1, :], e_sb[:, r0:r1, :],
                                                eneg[:, r0:r1, :], AX.add)
                    nc.scalar.activation(wpt[:, :TL, F:FH], e_sb[:, :TL, :], AF.Exp)
                    nc.scalar.activation(wpt[:, TLO:TLO + TH, F:FH],
                                         e_sb[:, TLO:TLO + TH, :], AF.Exp)
                    # w = h * p_expand
                    if cfg.bf16_rows:
                        glo_h = glo[:, :, :64].bitcast(mybir.dt.bfloat16)
                        ghi_h = ghi[:, :, :64].bitcast(mybir.dt.bfloat16)
                    else:
                        glo_h = glo[:, :, :F]
                        ghi_h = ghi[:, :, :F]
                    nc.vector.tensor_tensor(
                        wpt[:, :TL, :F].rearrange("p t (h c) -> p t h c", c=C),
                        glo_h[:, :TL, :].rearrange("p t (h c) -> p t h c", c=C),
                        wpt[:, :TL, F:FH].unsqueeze(3).broadcast_to([128, TL, H, C]),
                        AX.mult)
                    nc.vector.tensor_tensor(
                        wpt[:, TLO:TLO + TH, :F].rearrange("p t (h c) -> p t h c", c=C),
                        ghi_h[:, :TH, :].rearrange("p t (h c) -> p t h c", c=C),
                        wpt[:, TLO:TLO + TH, F:FH].unsqueeze(3).broadcast_to(
                            [128, TH, H, C]),
                        AX.mult)
                    # scatter matmuls
                    acc_ps = ps_pool.tile([128, FH], F32, tag="psB")
                    for i, t in enumerate(real_ts):
                        nc.tensor.matmul(acc_ps[:], sel[:, t, :], wpt[:, t, :],
                                         start=(i == 0),
                                         stop=(i == len(real_ts) - 1))
                    # divide + bias -> h_cur
                    s_sb = wp_pool.tile([128, H], F32, tag="s")
                    nc.vector.tensor_scalar(s_sb[:], acc_ps[:, F:FH], 1e-30, None,
                                            AX.max)
                    r_sb = wp_pool.tile([128, H], F32, tag="r")
                    nc.vector.reciprocal(r_sb[:], s_sb[:])
                    nc.vector.tensor_tensor(
                        h_cur[:, b, :].rearrange("p (h c) -> p h c", c=C),
                        acc_ps[:, :F].rearrange("p (h c) -> p h c", c=C),
                        r_sb[:].unsqueeze(2).broadcast_to([128, H, C]),
                        AX.mult)
                    nc.vector.tensor_tensor(h_cur[:, b, :], h_cur[:, b, :],
                                            bbc_sb[:], AX.add)

            def bn_elu_phase(li, g_sb, be_sb):
                bn_ps = ps1_pool.tile([1, 2 * F], F32, tag="ps1")
                for b in range(NB):
                    rhs = wp_pool.tile([128, 2 * F], F32, tag="bnrhs")
                    nc.vector.tensor_scalar(rhs[:, :F], h_cur[:, b, :],
                                            mask_sb[:, b].unsqueeze(1), None, AX.mult)
                    nc.scalar.activation(rhs[:, F:], rhs[:, :F], AF.Square)
                    nc.tensor.matmul(bn_ps[:], onescol[:], rhs[:],
                                     start=(b == 0), stop=(b == NB - 1))
                bn_sb = wp_pool.tile([1, 2 * F], F32, tag="bnrow")
                nc.vector.tensor_copy(bn_sb[:], bn_ps[:])
                nc.sync.dma_start(bn_in[li][:], bn_sb[:])
                nc.gpsimd.collective_compute(
                    "AllReduce", AX.add, replica_groups=rg,
                    ins=[bn_in[li][:]], outs=[bn_out[li][:]])
                st = wp_pool.tile([1, 2 * F], F32, tag="bnst")
                nc.sync.dma_start(st[:], bn_out[li][:])
                # mu = s/n ; var = ss/n - mu^2
                mu = wp_pool.tile([1, F], F32, tag="mu")
                nc.vector.tensor_scalar(mu[:], st[:, :F], 1.0 / cfg.n_real, None,
                                        AX.mult)
                var = wp_pool.tile([1, F], F32, tag="var")
                nc.vector.tensor_scalar(var[:], st[:, F:], 1.0 / cfg.n_real, None,
                                        AX.mult)
                mu2 = wp_pool.tile([1, F], F32, tag="mu2")
                nc.scalar.activation(mu2[:], mu[:], AF.Square)
                nc.vector.tensor_tensor(var[:], var[:], mu2[:], AX.subtract)
                # rstd = 1/sqrt(var+eps)
                nc.vector.tensor_scalar(var[:], var[:], cfg.eps, None, AX.add)
                sd = wp_pool.tile([1, F], F32, tag="sd")
                nc.scalar.activation(sd[:], var[:], AF.Sqrt)
                rstd = wp_pool.tile([1, F], F32, tag="rstd")
                nc.vector.reciprocal(rstd[:], sd[:])
                # scale = g*rstd ; shift = be - mu*scale
                ssrow = wp_pool.tile([1, 2 * F], F32, tag="ssrow")
                nc.vector.tensor_tensor(ssrow[:, :F], g_sb[:], rstd[:], AX.mult)
                musc = wp_pool.tile([1, F], F32, tag="musc")
                nc.vector.tensor_tensor(musc[:], mu[:], ssrow[:, :F], AX.mult)
                nc.vector.tensor_tensor(ssrow[:, F:], be_sb[:], musc[:], AX.subtract)
                # broadcast via K=1 matmul
                bc_ps = ps1_pool.tile([128, 2 * F], F32, tag="ps1")
                nc.tensor.matmul(bc_ps[:], onesrow[:], ssrow[:], start=True, stop=True)
                bc_sb = wp_pool.tile([128, 2 * F], F32, tag="bnbcsb")
                nc.vector.tensor_copy(bc_sb[:], bc_ps[:])
                # normalize + elu
                for b in range(NB):
                    nc.vector.tensor_tensor(h_cur[:, b, :], h_cur[:, b, :],
                                            bc_sb[:, :F], AX.mult)
                    nc.vector.tensor_tensor(h_cur[:, b, :], h_cur[:, b, :],
                                            bc_sb[:, F:], AX.add)
                    neg = wp_pool.tile([128, F], F32, tag="neg")
                    nc.vector.tensor_scalar(neg[:], h_cur[:, b, :], 0.0, None, AX.min)
                    ex = wp_pool.tile([128, F], F32, tag="ex")
                    nc.scalar.activation(ex[:], neg[:], AF.Exp)
                    nc.vector.tensor_scalar(h_cur[:, b, :], h_cur[:, b, :], 0.0, None,
                                            AX.max)
                    nc.vector.tensor_tensor(h_cur[:, b, :], h_cur[:, b, :], ex[:],
                                            AX.add)
                    nc.vector.tensor_scalar(h_cur[:, b, :], h_cur[:, b, :], -1.0,
                                            None, AX.add)

            # ================= program =================
            for _rep in range(cfg.repeat):
              dense_phase(0, w1e_sb)
              if cfg.stage >= 2:
                scatter_phase(0, b1bc_sb)
              if cfg.stage >= 3:
                bn_elu_phase(0, g1_sb, be1_sb)
              if cfg.stage >= 4:
                dense_phase(1, w2e_sb)
              if cfg.stage >= 5:
                scatter_phase(1, b2bc_sb)
                bn_elu_phase(1, g2_sb, be2_sb)
            if cfg.stage < 6:
                dbg = wp_pool.tile([cfg.k, 128], F32, tag="dbg")
                nc.vector.tensor_copy(dbg[:], h_cur[0:cfg.k, 0, :])
                nc.sync.dma_start(out_d[:, :128], dbg[:])
                nc.compile_marker = None  # no-op
            _full = cfg.stage >= 6
            # pooling
            try:
                pool_ps = ps1_pool.tile([128, GP], F32, tag="ps1")
                for b in range(NB if _full else 0):
                    gsel = wp_pool.tile([128, GP], F32, tag="gsel")
                    nc.vector.tensor_scalar(gsel[:], giota[:],
                                            bcol_sb[:, b].unsqueeze(1), None,
                                            AX.is_equal)
                    nc.tensor.matmul(pool_ps[:], h_cur[:, b, :], gsel[:],
                                     start=(b == 0), stop=(b == NB - 1))
                if not _full:
                    raise _SkipRest
                pool_sb = wp_pool.tile([128, GP], F32, tag="poolsb")
                nc.vector.tensor_copy(pool_sb[:], pool_ps[:])
                fc_ps = ps1_pool.tile([cfg.k, GP], F32, tag="ps1")
                nc.tensor.matmul(fc_ps[:], fcw_sb[:], pool_sb[:], start=True, stop=True)
                fc_sb = wp_pool.tile([cfg.k, GP], F32, tag="fcsb")
                nc.vector.tensor_copy(fc_sb[:], fc_ps[:])
                nc.sync.dma_start(fc_in[:], fc_sb[:])
                nc.gpsimd.collective_compute("AllReduce", AX.add, replica_groups=rg,
                                             ins=[fc_in[:]], outs=[fc_out[:]])
                fin = wp_pool.tile([cfg.k, GP], F32, tag="fin")
                nc.sync.dma_start(fin[:], fc_out[:])
                nc.vector.tensor_tensor(fin[:], fin[:], rcnt_sb[:], AX.mult)
                nc.vector.tensor_tensor(fin[:], fin[:], fcbbc_sb[:], AX.add)
                nc.sync.dma_start(out_d[:], fin[:])
            except _SkipRest:
                pass

    nc.compile()
    return nc


# ---------------------------------------------------------------------------
# harness entry point: full inputs in, full output out
# ---------------------------------------------------------------------------

_NC_CACHE = {}


def kernel(**inputs):
    """Full-input GAT forward on 8 NeuronCores. Returns [500, 6] float32."""
    from concourse.bass_utils import run_bass_kernel_spmd

    cfg = Cfg(bf16_rows=True, bf16_mm=True)
    in_maps = prep_inputs(cfg, inputs)
    key = (cfg.tlo, cfg.thi, cfg.bf16_rows, cfg.bf16_mm)
    if key not in _NC_CACHE:
        _NC_CACHE[key] = build_nc(cfg)
    nc = _NC_CACHE[key]
    res = run_bass_kernel_spmd(nc, in_maps, core_ids=list(range(cfg.ncores)))
    out = res.results[0]["out"]
    return np.ascontiguousarray(out[:, :cfg.g].T).astype(np.float32)



# revision 11
# speedup vs baseline: 1.1730x; 1.1730x over previous
"""GAT kernel for TRN2: host prep + Bass program builder + numpy model.

Sharding: nodes (and their in-edges) partitioned across cores by contiguous
shard; per dst-block-of-128 selector-matmul scatter; edge gathers of packed
table rows [h(128f32) | al(4f32) | pad] = 768B via gpsimd dma_gather with a
lo/hi table split (int16 index limit); inter-layer AllGather of the table;
BN via AllReduce of partial sums; pooling via transposed graph-selector
matmul; tiny FC + final AllReduce.
"""
from dataclasses import dataclass

import numpy as np

import concourse.bacc as bacc
import concourse.bass as bass
import concourse.mybir as mybir
import concourse.tile as tile
from concourse import library_config

F32 = mybir.dt.float32
I16 = mybir.dt.int16
I8 = mybir.dt.int8
AX = mybir.AluOpType
AF = mybir.ActivationFunctionType


class _SkipRest(Exception):
    pass



@dataclass
class Cfg:
    ncores: int = 8
    n_real: int = 50000       # real nodes
    np_: int = 50176          # padded nodes (multiple of ncores*128)
    e_raw: int = 800000       # edges before self loops
    g: int = 500              # graphs
    gp: int = 512             # padded graphs (pool matmul free dim)
    tlo: int = 0              # max tiles per block from lo table (computed in prep)
    thi: int = 0
    tlo_b: tuple = ()         # per-block lo tile counts (max over cores)
    thi_b: tuple = ()
    f: int = 128              # features (in = out = 128)
    h: int = 4
    c: int = 32
    k: int = 6
    eps: float = 1e-5
    rowf: int = 192           # table row floats (768B); 128 when bf16_rows
    stage: int = 9            # debug: how much of the program to emit
    repeat: int = 1           # timing: execute the whole body N times
    bf16_rows: bool = False   # pack h as bf16 in 512B table rows
    bf16_mm: bool = False     # bf16 selectors + weighted features (FWL matmuls)
    single_packet: bool = True  # coalesce gather descriptor stream packets
    gather_chunks: int = 2    # split each lo/hi gather into N queue-parallel chunks
    neg_slope: float = 0.2

    @property
    def shard(self):
        return self.np_ // self.ncores

    @property
    def nblk(self):
        return self.shard // 128

    @property
    def half(self):
        return self.np_ // 2

    @property
    def tpb(self):
        return self.tlo + self.thi


def fold_attn(a, H, C):
    A = np.zeros((H * C, H), np.float32)
    for h in range(H):
        A[h * C:(h + 1) * C, h] = a[h]
    return A


def pack_idx16(idx):
    """int array [n] (n % 128 == 0) -> [128, n//16] int16 dma_gather layout."""
    n = len(idx)
    arr = np.zeros((16, n // 16), dtype=np.int16)
    arr[np.arange(n) % 16, np.arange(n) // 16] = idx
    return np.tile(arr, (8, 1))


def prep_edges(cfg: Cfg, edge_index):
    """Returns srcp/dstp [ncores, nblk, tpb, 128] (int64; dst sentinel 999)
    and sets cfg.tlo/thi."""
    n, npd, sh = cfg.n_real, cfg.np_, cfg.shard
    src = np.concatenate([edge_index[0], np.arange(n)]).astype(np.int64)
    dst = np.concatenate([edge_index[1], np.arange(n)]).astype(np.int64)
    order = np.argsort(dst // sh, kind="stable")
    buckets = [[None] * cfg.nblk for _ in range(cfg.ncores)]
    tlo_b = [1] * cfg.nblk
    thi_b = [1] * cfg.nblk
    for ci in range(cfg.ncores):
        m = (dst // sh) == ci
        s, d = src[m], dst[m] - ci * sh
        for b in range(cfg.nblk):
            mb = (d // 128) == b
            sb, db = s[mb], d[mb] % 128
            lo = sb < cfg.half
            buckets[ci][b] = ((sb[lo], db[lo]), (sb[~lo] - cfg.half, db[~lo]))
            tlo_b[b] = max(tlo_b[b], -(-len(sb[lo]) // 128))
            thi_b[b] = max(thi_b[b], -(-int((~lo).sum()) // 128))
    cfg.tlo, cfg.thi = max(tlo_b), max(thi_b)
    cfg.tlo_b, cfg.thi_b = tuple(tlo_b), tuple(thi_b)
    tlo, thi = cfg.tlo, cfg.thi
    tpb = cfg.tpb
    srcp = np.zeros((cfg.ncores, cfg.nblk, tpb, 128), np.int64)
    dstp = np.full((cfg.ncores, cfg.nblk, tpb, 128), 999, np.int64)
    for ci in range(cfg.ncores):
        for b in range(cfg.nblk):
            (slo, dlo), (shi, dhi) = buckets[ci][b]
            srcp[ci, b, :tlo].flat[: len(slo)] = slo
            dstp[ci, b, :tlo].flat[: len(dlo)] = dlo
            srcp[ci, b, tlo:].flat[: len(shi)] = shi
            dstp[ci, b, tlo:].flat[: len(dhi)] = dhi
    return srcp, dstp


def prep_inputs(cfg: Cfg, inputs):
    """Build per-core in_maps (list of dicts)."""
    H, C, F = cfg.h, cfg.c, cfg.f
    srcp, dstp = prep_edges(cfg, inputs["edge_index"])
    tlo, thi, tpb = cfg.tlo, cfg.thi, cfg.tpb

    xpad = np.zeros((cfg.np_, F), np.float32)
    xpad[: cfg.n_real] = inputs["x"]

    W1e = np.concatenate(
        [inputs["W1"], inputs["W1"] @ fold_attn(inputs["a_src1"], H, C),
         inputs["W1"] @ fold_attn(inputs["a_dst1"], H, C)], axis=1)  # [F,136]
    W2e = np.concatenate(
        [inputs["W2"], inputs["W2"] @ fold_attn(inputs["a_src2"], H, C),
         inputs["W2"] @ fold_attn(inputs["a_dst2"], H, C)], axis=1)

    batch = np.asarray(inputs["batch"]).astype(np.int64)
    batch_pad = np.full(cfg.np_, 999, np.int64)
    batch_pad[: cfg.n_real] = batch
    cnt = np.bincount(batch, minlength=cfg.gp).astype(np.float32)
    rcnt = (1.0 / np.maximum(cnt, 1.0)).astype(np.float32)

    shared = dict(
        w1e=W1e.astype(np.float32), w2e=W2e.astype(np.float32),
        b1bc=np.tile(inputs["b1"][None, :], (128, 1)).astype(np.float32),
        b2bc=np.tile(inputs["b2"][None, :], (128, 1)).astype(np.float32),
        g1row=inputs["g1"][None, :].astype(np.float32),
        be1row=inputs["be1"][None, :].astype(np.float32),
        g2row=inputs["g2"][None, :].astype(np.float32),
        be2row=inputs["be2"][None, :].astype(np.float32),
        fcw=inputs["fcW"].astype(np.float32),
        fcbbc=np.tile(inputs["fcb"][:, None], (1, cfg.gp)).astype(np.float32),
        rcntbc=np.tile(rcnt[None, :], (cfg.k, 1)).astype(np.float32),
        ident=np.eye(128, dtype=np.float32),
        giota=np.tile(np.arange(cfg.gp, dtype=np.float32)[None, :], (128, 1)),
        onescol=np.ones((128, 1), np.float32),
        onesrow=np.ones((1, 128), np.float32),
    )

    in_maps = []
    iota128 = np.arange(128, dtype=np.int64)
    seldt = mybir.dt.np(mybir.dt.bfloat16) if cfg.bf16_mm else np.float32
    for ci in range(cfg.ncores):
        idx_lo = np.zeros((cfg.nblk, 128, tlo * 8), np.int16)
        idx_hi = np.zeros((cfg.nblk, 128, thi * 8), np.int16)
        for b in range(cfg.nblk):
            idx_lo[b] = pack_idx16(srcp[ci, b, :tlo].reshape(-1))
            idx_hi[b] = pack_idx16(srcp[ci, b, tlo:].reshape(-1))
        dst_blk = dstp[ci]  # [nblk, tpb, 128] (999 pad)
        # selp[b, p, t, j] = (dst[b, t, p] == j): stationary scatter selector
        selp = (dst_blk.transpose(0, 2, 1)[:, :, :, None]
                == iota128[None, None, None, :]).astype(seldt)
        # selT[b, p, t*128+k] = (dst[b, t, k] == p): arx gather selector
        selT = (dst_blk[:, None, :, :]
                == iota128[None, :, None, None]).astype(seldt)
        selT = selT.reshape(cfg.nblk, 128, tpb * 128)
        sl = slice(ci * cfg.shard, (ci + 1) * cfg.shard)
        nm = np.zeros((128, cfg.nblk), np.float32)
        bc = np.zeros((128, cfg.nblk), np.float32)
        ids = np.arange(ci * cfg.shard, (ci + 1) * cfg.shard)
        nm[:] = (ids.reshape(cfg.nblk, 128).T < cfg.n_real)
        bc[:] = batch_pad[ids].reshape(cfg.nblk, 128).T.astype(np.float32)
        xs = xpad[sl].reshape(cfg.nblk, 128, F).transpose(1, 0, 2)
        in_maps.append(dict(
            x_shard=np.ascontiguousarray(xs).reshape(128, cfg.nblk * F),
            idx_lo=idx_lo, idx_hi=idx_hi,
            selp=selp,
            selT=selT,
            node_mask=nm, batchcol=bc,
            **shared,
        ))
    return in_maps


# ---------------------------------------------------------------------------
# numpy model (for validation at any cfg)
# ---------------------------------------------------------------------------

def numpy_forward(cfg: Cfg, inputs):
    H, C, F = cfg.h, cfg.c, cfg.f
    srcp, dstp = prep_edges(cfg, inputs["edge_index"])
    xpad = np.zeros((cfg.np_, F), np.float32)
    xpad[: cfg.n_real] = inputs["x"]

    def layer(xp, W, asrc, adst, b):
        We = np.concatenate([W, W @ fold_attn(asrc, H, C), W @ fold_attn(adst, H, C)], 1)
        tab = xp @ We
        out = np.zeros((cfg.np_, F), np.float32)
        for ci in range(cfg.ncores):
            for bi in range(cfg.nblk):
                base = ci * cfg.shard + bi * 128
                acc = np.zeros((128, F + H), np.float32)
                ar_blk = tab[base: base + 128, F + H: F + 2 * H]
                for t in range(cfg.tpb):
                    s = srcp[ci, bi, t] + (cfg.half if t >= cfg.tlo else 0)
                    dl = dstp[ci, bi, t]
                    grow = tab[s]
                    sel = (dl[:, None] == np.arange(128)[None, :]).astype(np.float32)
                    e = grow[:, F:F + H] + sel @ ar_blk
                    e = np.where(e > 0, e, cfg.neg_slope * e).astype(np.float32)
                    p = np.exp(e).astype(np.float32)
                    w = grow[:, :F] * np.repeat(p, C, 1)
                    acc += sel.T @ np.concatenate([w, p], 1)
                ssum = np.maximum(np.repeat(acc[:, F:], C, 1), 1e-30)
                out[base:base + 128] = acc[:, :F] / ssum + b
        return out

    def bn_elu(hh, g, be):
        s, ss = hh[:cfg.n_real].sum(0), (hh[:cfg.n_real] ** 2).sum(0)
        mu = s / cfg.n_real
        var = ss / cfg.n_real - mu ** 2
        sc = g / np.sqrt(var + cfg.eps)
        sh = be - mu * sc
        y = hh * sc + sh
        return (np.where(y > 0, y, np.exp(np.minimum(y, 0)) - 1)).astype(np.float32)

    h1 = layer(xpad, inputs["W1"], inputs["a_src1"], inputs["a_dst1"], inputs["b1"])
    h1n = bn_elu(h1, inputs["g1"], inputs["be1"])
    h2 = layer(h1n, inputs["W2"], inputs["a_src2"], inputs["a_dst2"], inputs["b2"])
    h2n = bn_elu(h2, inputs["g2"], inputs["be2"])

    batch = np.asarray(inputs["batch"]).astype(np.int64)
    gsel = np.zeros((cfg.n_real, cfg.gp), np.float32)
    gsel[np.arange(cfg.n_real), batch] = 1.0
    pooled = h2n[:cfg.n_real].T @ gsel
    fc = inputs["fcW"].T.astype(np.float32) @ pooled
    cnt = np.bincount(batch, minlength=cfg.gp).astype(np.float32)
    fc = fc / np.maximum(cnt, 1.0)[None, :] + inputs["fcb"][:, None]
    return fc[:, :cfg.g].T  # [g, k]


# ---------------------------------------------------------------------------
# Bass program
# ---------------------------------------------------------------------------

def build_nc(cfg: Cfg):
    NB, TPB, TLO, THI = cfg.nblk, cfg.tpb, cfg.tlo, cfg.thi
    F, H, C, RF = cfg.f, cfg.h, cfg.c, cfg.rowf
    FH = F + H
    SH = cfg.shard
    GP = cfg.gp

    if cfg.bf16_mm:
        assert cfg.bf16_rows, "bf16_mm requires bf16_rows"
    if cfg.bf16_rows:
        cfg.rowf = 128
        RF = 128
    ALO = 64 if cfg.bf16_rows else F   # f32-slot offset of al in a row
    BF = mybir.dt.bfloat16
    MMDT = BF if cfg.bf16_mm else F32
    nc = bacc.Bacc("TRN2", target_bir_lowering=False, debug=False,
                   num_devices=cfg.ncores, num_swdge_queues=4)

    def ext(name, shape, dtype=F32):
        return nc.dram_tensor(name, shape, dtype, kind="ExternalInput")

    x_shard = ext("x_shard", [128, NB * F])
    idx_lo = ext("idx_lo", [NB, 128, TLO * 8], I16)
    idx_hi = ext("idx_hi", [NB, 128, THI * 8], I16)
    selp_d = ext("selp", [NB, 128, TPB, 128], MMDT)
    selT_d = ext("selT", [NB, 128, TPB * 128], MMDT)
    node_mask = ext("node_mask", [128, NB])
    batchcol = ext("batchcol", [128, NB])
    w1e = ext("w1e", [F, F + 2 * H])
    w2e = ext("w2e", [F, F + 2 * H])
    b1bc = ext("b1bc", [128, F])
    b2bc = ext("b2bc", [128, F])
    g1row = ext("g1row", [1, F])
    be1row = ext("be1row", [1, F])
    g2row = ext("g2row", [1, F])
    be2row = ext("be2row", [1, F])
    fcw = ext("fcw", [F, cfg.k])
    fcbbc = ext("fcbbc", [cfg.k, GP])
    rcntbc = ext("rcntbc", [cfg.k, GP])
    ident_d = ext("ident", [128, 128])
    giota_d = ext("giota", [128, GP])
    onescol_d = ext("onescol", [128, 1])
    onesrow_d = ext("onesrow", [1, 128])

    out_d = nc.dram_tensor("out", [cfg.k, GP], F32, kind="ExternalOutput")

    rg = [list(range(cfg.ncores))]
    shared_as = "Shared" if cfg.ncores > 4 else "Local"

    with tile.TileContext(nc) as tc:
        with (
            tc.tile_pool(name="dram", bufs=1, space="DRAM") as dpool,
            tc.tile_pool(name="persist", bufs=1) as pp,
            tc.tile_pool(name="consts", bufs=1) as cp,
            tc.tile_pool(name="work", bufs=3) as wp_pool,
            tc.tile_pool(name="gath", bufs=4) as gp_pool,
            tc.tile_pool(name="psum", bufs=3, space="PSUM") as ps_pool,
            tc.tile_pool(name="psum1", bufs=1, space="PSUM") as ps1_pool,
        ):
            nc.gpsimd.load_library(library_config.mlp)

            # ---- persistent SBUF ----
            h_cur = pp.tile([128, NB, F], F32)          # shard activations
            ar_sb = pp.tile([128, NB, H], MMDT)
            mask_sb = pp.tile([128, NB], F32)
            bcol_sb = pp.tile([128, NB], F32)

            # ---- constants ----
            w1e_sb = cp.tile([128, F + 2 * H], F32)
            w2e_sb = cp.tile([128, F + 2 * H], F32)
            b1bc_sb = cp.tile([128, F], F32)
            b2bc_sb = cp.tile([128, F], F32)
            ident = cp.tile([128, 128], F32)
            giota = cp.tile([128, GP], F32)
            onescol = cp.tile([128, 1], F32)
            onesrow = cp.tile([1, 128], F32)
            g1_sb = cp.tile([1, F], F32)
            be1_sb = cp.tile([1, F], F32)
            g2_sb = cp.tile([1, F], F32)
            be2_sb = cp.tile([1, F], F32)
            fcw_sb = cp.tile([128, cfg.k], F32)
            fcbbc_sb = cp.tile([cfg.k, GP], F32)
            rcnt_sb = cp.tile([cfg.k, GP], F32)

            for sb, d in [(w1e_sb, w1e), (w2e_sb, w2e), (b1bc_sb, b1bc),
                          (b2bc_sb, b2bc), (ident, ident_d),
                          (giota, giota_d), (onescol, onescol_d),
                          (onesrow, onesrow_d), (g1_sb, g1row), (be1_sb, be1row),
                          (g2_sb, g2row), (be2_sb, be2row), (fcw_sb, fcw),
                          (fcbbc_sb, fcbbc), (rcnt_sb, rcntbc),
                          (mask_sb, node_mask), (bcol_sb, batchcol),
                          (h_cur, x_shard)]:
                nc.sync.dma_start(sb[:], d[:])

            # ---- DRAM internals ----
            ht_in = [dpool.tile([SH, RF], F32, name=f"ht{i}_in") for i in (1, 2)]
            ht = [dpool.tile([cfg.np_, RF], F32, addr_space=shared_as, name=f"ht{i}")
                  for i in (1, 2)]
            bn_in = [dpool.tile([1, 2 * F], F32, name=f"bn{i}_in") for i in (1, 2)]
            bn_out = [dpool.tile([1, 2 * F], F32, addr_space=shared_as, name=f"bn{i}_out")
                      for i in (1, 2)]
            fc_in = dpool.tile([cfg.k, GP], F32)
            fc_out = dpool.tile([cfg.k, GP], F32, addr_space=shared_as)

            # ================= helper phases =================

            def dense_phase(li, wext_sb):
                """h_cur -> table rows (HT_in) + ar_sb; then AllGather."""
                for b in range(NB):
                    tr_ps = ps_pool.tile([128, 128], F32, tag="psA")
                    nc.tensor.transpose(tr_ps[:], h_cur[:, b, :], ident[:])
                    xT = wp_pool.tile([128, 128], F32, tag="xT")
                    nc.vector.tensor_copy(xT[:], tr_ps[:])
                    dp_ps = ps_pool.tile([128, F + 2 * H], F32, tag="psB")
                    nc.tensor.matmul(dp_ps[:], xT[:], wext_sb[:], start=True, stop=True)
                    row = wp_pool.tile([128, RF], F32, tag="row")
                    if cfg.bf16_rows:
                        nc.vector.tensor_copy(
                            row[:, :64].bitcast(mybir.dt.bfloat16), dp_ps[:, :F])
                        nc.vector.tensor_copy(row[:, 64:64 + H], dp_ps[:, F:FH])
                        nc.vector.memset(row[:, 64 + H:], 0.0)
                    else:
                        nc.vector.tensor_copy(row[:, :FH], dp_ps[:, :FH])
                        nc.vector.memset(row[:, FH:], 0.0)
                    nc.vector.tensor_copy(ar_sb[:, b, :], dp_ps[:, FH:FH + H])
                    nc.sync.dma_start(
                        ht_in[li][b * 128:(b + 1) * 128, :],
                        row[:])
                nc.gpsimd.collective_compute(
                    "AllGather", AX.bypass, replica_groups=rg,
                    ins=[ht_in[li][:]], outs=[ht[li][:]])

            def scatter_phase(li, bbc_sb):
                """edge phase: gathers + selector matmuls -> h_cur (+bias)."""
                tab = ht[li]
                for b in range(NB):
                    TL, TH = cfg.tlo_b[b], cfg.thi_b[b]
                    ilo_t = gp_pool.tile([128, TLO * 8], I16, tag="ilo")
                    ihi_t = gp_pool.tile([128, THI * 8], I16, tag="ihi")
                    nc.sync.dma_start(ilo_t[:, :TL * 8], idx_lo[b, :, :TL * 8])
                    nc.sync.dma_start(ihi_t[:, :TH * 8], idx_hi[b, :, :TH * 8])
                    glo = gp_pool.tile([128, TLO, RF], F32, tag="glo")
                    ghi = gp_pool.tile([128, THI, RF], F32, tag="ghi")
                    # chunked queue-parallel gathers: lo on queues 0..1, hi 2..3
                    def gather_chunks(dst_t, src_ap, idxs_t, T, qbase):
                        nch = min(cfg.gather_chunks, T)
                        bounds = [T * i // nch for i in range(nch + 1)]
                        for c in range(nch):
                            a, z = bounds[c], bounds[c + 1]
                            if z <= a:
                                continue
                            nc.gpsimd.dma_gather(
                                out_ap=dst_t[:, a:z, :], in_ap=src_ap,
                                idxs_ap=idxs_t[:, a * 8:z * 8],
                                num_idxs=(z - a) * 128,
                                num_idxs_reg=(z - a) * 128, elem_size=RF,
                                queue_num=qbase + c,
                                single_packet=cfg.single_packet)
                    gather_chunks(glo, tab[:cfg.half, :], ilo_t, TL, 0)
                    gather_chunks(ghi, tab[cfg.half:, :], ihi_t, TH, 2)
                    # selectors (host-precomputed, DMA-streamed)
                    sel = gp_pool.tile([128, TPB, 128], MMDT, tag="sel")
                    selT = gp_pool.tile([128, TPB * 128], MMDT, tag="selT")
                    nc.sync.dma_start(sel[:, :TL, :], selp_d[b, :, :TL, :])
                    nc.sync.dma_start(sel[:, TLO:TLO + TH, :],
                                      selp_d[b, :, TLO:TLO + TH, :])
                    nc.sync.dma_start(selT[:, :TL * 128], selT_d[b, :, :TL * 128])
                    nc.sync.dma_start(selT[:, TLO * 128:(TLO + TH) * 128],
                                      selT_d[b, :, TLO * 128:(TLO + TH) * 128])
                    # ar expand per tile
                    arx_ps = ps_pool.tile([128, TPB, H], F32, tag="psA")
                    real_ts = list(range(TL)) + list(range(TLO, TLO + TH))
                    for t in real_ts:
                        nc.tensor.matmul(arx_ps[:, t, :],
                                         selT[:, t * 128:(t + 1) * 128],
                                         ar_sb[:, b, :], start=True, stop=True)
                    # e = al + ar ; lrelu = max(e, 0.2e); exp -> p
                    wpt = wp_pool.tile([128, TPB, FH], MMDT, tag="wpt")
                    e_sb = wp_pool.tile([128, TPB, H], F32, tag="e")
                    nc.vector.tensor_tensor(e_sb[:, :TL, :],
                                            glo[:, :TL, ALO:ALO + H],
                                            arx_ps[:, :TL, :], AX.add)
                    nc.vector.tensor_tensor(e_sb[:, TLO:TLO + TH, :],
                                            ghi[:, :TH, ALO:ALO + H],
                                            arx_ps[:, TLO:TLO + TH, :], AX.add)
                    eneg = wp_pool.tile([128, TPB, H], F32, tag="eneg")
                    for r0, r1 in ((0, TL), (TLO, TLO + TH)):
                        nc.vector.tensor_scalar(eneg[:, r0:r1, :], e_sb[:, r0:r1, :],
                                                cfg.neg_slope, None, AX.mult)
                        nc.vector.tensor_tensor(e_sb[:, r0:r1, :], e_sb[:, r0:r1, :],
                                                eneg[:, r0:r1, :], AX.max)
                    nc.scalar.activation(wpt[:, :TL, F:FH], e_sb[:, :TL, :], AF.Exp)
                    nc.scalar.activation(wpt[:, TLO:TLO + TH, F:FH],
                                         e_sb[:, TLO:TLO + TH, :], AF.Exp)
                    # w = h * p_expand
                    if cfg.bf16_rows:
                        glo_h = glo[:, :, :64].bitcast(mybir.dt.bfloat16)
                        ghi_h = ghi[:, :, :64].bitcast(mybir.dt.bfloat16)
                    else:
                        glo_h = glo[:, :, :F]
                        ghi_h = ghi[:, :, :F]
                    nc.vector.tensor_tensor(
                        wpt[:, :TL, :F].rearrange("p t (h c) -> p t h c", c=C),
                        glo_h[:, :TL, :].rearrange("p t (h c) -> p t h c", c=C),
                        wpt[:, :TL, F:FH].unsqueeze(3).broadcast_to([128, TL, H, C]),
                        AX.mult)
                    nc.vector.tensor_tensor(
                        wpt[:, TLO:TLO + TH, :F].rearrange("p t (h c) -> p t h c", c=C),
                        ghi_h[:, :TH, :].rearrange("p t (h c) -> p t h c", c=C),
                        wpt[:, TLO:TLO + TH, F:FH].unsqueeze(3).broadcast_to(
                            [128, TH, H, C]),
                        AX.mult)
                    # scatter matmuls
                    acc_ps = ps_pool.tile([128, FH], F32, tag="psB")
                    for i, t in enumerate(real_ts):
                        nc.tensor.matmul(acc_ps[:], sel[:, t, :], wpt[:, t, :],
                                         start=(i == 0),
                                         stop=(i == len(real_ts) - 1))
                    # divide + bias -> h_cur
                    s_sb = wp_pool.tile([128, H], F32, tag="s")
                    nc.vector.tensor_scalar(s_sb[:], acc_ps[:, F:FH], 1e-30, None,
                                            AX.max)
                    r_sb = wp_pool.tile([128, H], F32, tag="r")
                    nc.vector.reciprocal(r_sb[:], s_sb[:])
                    nc.vector.tensor_tensor(
                        h_cur[:, b, :].rearrange("p (h c) -> p h c", c=C),
                        acc_ps[:, :F].rearrange("p (h c) -> p h c", c=C),
                        r_sb[:].unsqueeze(2).broadcast_to([128, H, C]),
                        AX.mult)
                    nc.vector.tensor_tensor(h_cur[:, b, :], h_cur[:, b, :],
                                            bbc_sb[:], AX.add)

            def bn_elu_phase(li, g_sb, be_sb):
                bn_ps = ps1_pool.tile([1, 2 * F], F32, tag="ps1")
                for b in range(NB):
                    rhs = wp_pool.tile([128, 2 * F], F32, tag="bnrhs")
                    nc.vector.tensor_scalar(rhs[:, :F], h_cur[:, b, :],
                                            mask_sb[:, b].unsqueeze(1), None, AX.mult)
                    nc.scalar.activation(rhs[:, F:], rhs[:, :F], AF.Square)
                    nc.tensor.matmul(bn_ps[:], onescol[:], rhs[:],
                                     start=(b == 0), stop=(b == NB - 1))
                bn_sb = wp_pool.tile([1, 2 * F], F32, tag="bnrow")
                nc.vector.tensor_copy(bn_sb[:], bn_ps[:])
                nc.sync.dma_start(bn_in[li][:], bn_sb[:])
                nc.gpsimd.collective_compute(
                    "AllReduce", AX.add, replica_groups=rg,
                    ins=[bn_in[li][:]], outs=[bn_out[li][:]])
                st = wp_pool.tile([1, 2 * F], F32, tag="bnst")
                nc.sync.dma_start(st[:], bn_out[li][:])
                # mu = s/n ; var = ss/n - mu^2
                mu = wp_pool.tile([1, F], F32, tag="mu")
                nc.vector.tensor_scalar(mu[:], st[:, :F], 1.0 / cfg.n_real, None,
                                        AX.mult)
                var = wp_pool.tile([1, F], F32, tag="var")
                nc.vector.tensor_scalar(var[:], st[:, F:], 1.0 / cfg.n_real, None,
                                        AX.mult)
                mu2 = wp_pool.tile([1, F], F32, tag="mu2")
                nc.scalar.activation(mu2[:], mu[:], AF.Square)
                nc.vector.tensor_tensor(var[:], var[:], mu2[:], AX.subtract)
                # rstd = 1/sqrt(var+eps)
                nc.vector.tensor_scalar(var[:], var[:], cfg.eps, None, AX.add)
                sd = wp_pool.tile([1, F], F32, tag="sd")
                nc.scalar.activation(sd[:], var[:], AF.Sqrt)
                rstd = wp_pool.tile([1, F], F32, tag="rstd")
                nc.vector.reciprocal(rstd[:], sd[:])
                # scale = g*rstd ; shift = be - mu*scale
                ssrow = wp_pool.tile([1, 2 * F], F32, tag="ssrow")
                nc.vector.tensor_tensor(ssrow[:, :F], g_sb[:], rstd[:], AX.mult)
                musc = wp_pool.tile([1, F], F32, tag="musc")
                nc.vector.tensor_tensor(musc[:], mu[:], ssrow[:, :F], AX.mult)
                nc.vector.tensor_tensor(ssrow[:, F:], be_sb[:], musc[:], AX.subtract)
                # broadcast via K=1 matmul
                bc_ps = ps1_pool.tile([128, 2 * F], F32, tag="ps1")
                nc.tensor.matmul(bc_ps[:], onesrow[:], ssrow[:], start=True, stop=True)
                bc_sb = wp_pool.tile([128, 2 * F], F32, tag="bnbcsb")
                nc.vector.tensor_copy(bc_sb[:], bc_ps[:])
                # normalize + elu
                for b in range(NB):
                    nc.vector.tensor_tensor(h_cur[:, b, :], h_cur[:, b, :],
                                            bc_sb[:, :F], AX.mult)
                    nc.vector.tensor_tensor(h_cur[:, b, :], h_cur[:, b, :],
                                            bc_sb[:, F:], AX.add)
                    neg = wp_pool.tile([128, F], F32, tag="neg")
                    nc.vector.tensor_scalar(neg[:], h_cur[:, b, :], 0.0, None, AX.min)
                    ex = wp_pool.tile([128, F], F32, tag="ex")
                    nc.scalar.activation(ex[:], neg[:], AF.Exp)
                    nc.vector.tensor_scalar(h_cur[:, b, :], h_cur[:, b, :], 0.0, None,
                                            AX.max)
                    nc.vector.tensor_tensor(h_cur[:, b, :], h_cur[:, b, :], ex[:],
                                            AX.add)
                    nc.vector.tensor_scalar(h_cur[:, b, :], h_cur[:, b, :], -1.0,
                                            None, AX.add)

            # ================= program =================
            for _rep in range(cfg.repeat):
              dense_phase(0, w1e_sb)
              if cfg.stage >= 2:
                scatter_phase(0, b1bc_sb)
              if cfg.stage >= 3:
                bn_elu_phase(0, g1_sb, be1_sb)
              if cfg.stage >= 4:
                dense_phase(1, w2e_sb)
              if cfg.stage >= 5:
                scatter_phase(1, b2bc_sb)
                bn_elu_phase(1, g2_sb, be2_sb)
            if cfg.stage < 6:
                dbg = wp_pool.tile([cfg.k, 128], F32, tag="dbg")
                nc.vector.tensor_copy(dbg[:], h_cur[0:cfg.k, 0, :])
                nc.sync.dma_start(out_d[:, :128], dbg[:])
                nc.compile_marker = None  # no-op
            _full = cfg.stage >= 6
            # pooling
            try:
                pool_ps = ps1_pool.tile([128, GP], F32, tag="ps1")
                for b in range(NB if _full else 0):
                    gsel = wp_pool.tile([128, GP], F32, tag="gsel")
                    nc.vector.tensor_scalar(gsel[:], giota[:],
                                            bcol_sb[:, b].unsqueeze(1), None,
                                            AX.is_equal)
                    nc.tensor.matmul(pool_ps[:], h_cur[:, b, :], gsel[:],
                                     start=(b == 0), stop=(b == NB - 1))
                if not _full:
                    raise _SkipRest
                pool_sb = wp_pool.tile([128, GP], F32, tag="poolsb")
                nc.vector.tensor_copy(pool_sb[:], pool_ps[:])
                fc_ps = ps1_pool.tile([cfg.k, GP], F32, tag="ps1")
                nc.tensor.matmul(fc_ps[:], fcw_sb[:], pool_sb[:], start=True, stop=True)
                fc_sb = wp_pool.tile([cfg.k, GP], F32, tag="fcsb")
                nc.vector.tensor_copy(fc_sb[:], fc_ps[:])
                nc.sync.dma_start(fc_in[:], fc_sb[:])
                nc.gpsimd.collective_compute("AllReduce", AX.add, replica_groups=rg,
                                             ins=[fc_in[:]], outs=[fc_out[:]])
                fin = wp_pool.tile([cfg.k, GP], F32, tag="fin")
                nc.sync.dma_start(fin[:], fc_out[:])
                nc.vector.tensor_tensor(fin[:], fin[:], rcnt_sb[:], AX.mult)
                nc.vector.tensor_tensor(fin[:], fin[:], fcbbc_sb[:], AX.add)
                nc.sync.dma_start(out_d[:], fin[:])
            except _SkipRest:
                pass

    nc.compile()
    return nc


# ---------------------------------------------------------------------------
# harness entry point: full inputs in, full output out
# ---------------------------------------------------------------------------

_NC_CACHE = {}


def kernel(**inputs):
    """Full-input GAT forward on 8 NeuronCores. Returns [500, 6] float32."""
    from concourse.bass_utils import run_bass_kernel_spmd

    cfg = Cfg(bf16_rows=True, bf16_mm=True)
    in_maps = prep_inputs(cfg, inputs)
    key = (cfg.tlo, cfg.thi, cfg.bf16_rows, cfg.bf16_mm)
    if key not in _NC_CACHE:
        _NC_CACHE[key] = build_nc(cfg)
    nc = _NC_CACHE[key]
    res = run_bass_kernel_spmd(nc, in_maps, core_ids=list(range(cfg.ncores)))
    out = res.results[0]["out"]
    return np.ascontiguousarray(out[:, :cfg.g].T).astype(np.float32)



# revision 19
# speedup vs baseline: 1.2659x; 1.0792x over previous
"""GAT kernel for TRN2: host prep + Bass program builder + numpy model.

Sharding: nodes (and their in-edges) partitioned across cores by contiguous
shard; per dst-block-of-128 selector-matmul scatter; edge gathers of packed
table rows [h(128f32) | al(4f32) | pad] = 768B via gpsimd dma_gather with a
lo/hi table split (int16 index limit); inter-layer AllGather of the table;
BN via AllReduce of partial sums; pooling via transposed graph-selector
matmul; tiny FC + final AllReduce.
"""
from dataclasses import dataclass

import numpy as np

import concourse.bacc as bacc
import concourse.bass as bass
import concourse.mybir as mybir
import concourse.tile as tile
from concourse import library_config

F32 = mybir.dt.float32
I16 = mybir.dt.int16
I8 = mybir.dt.int8
AX = mybir.AluOpType
AF = mybir.ActivationFunctionType


class _SkipRest(Exception):
    pass



@dataclass
class Cfg:
    ncores: int = 8
    n_real: int = 50000       # real nodes
    np_: int = 50176          # padded nodes (multiple of ncores*128)
    e_raw: int = 800000       # edges before self loops
    g: int = 500              # graphs
    gp: int = 512             # padded graphs (pool matmul free dim)
    tlo: int = 0              # max tiles per block from lo table (computed in prep)
    thi: int = 0
    tlo_b: tuple = ()         # per-block lo tile counts (max over cores)
    thi_b: tuple = ()
    f: int = 128              # features (in = out = 128)
    h: int = 4
    c: int = 32
    k: int = 6
    eps: float = 1e-5
    rowf: int = 192           # table row floats (768B); 128 when bf16_rows
    stage: int = 9            # debug: how much of the program to emit
    repeat: int = 1           # timing: execute the whole body N times
    bf16_rows: bool = False   # pack h as bf16 in 512B table rows
    bf16_mm: bool = False     # bf16 selectors + weighted features (FWL matmuls)
    single_packet: bool = True  # coalesce gather descriptor stream packets
    gather_chunks: int = 2    # split each lo/hi gather into N queue-parallel chunks
    neg_slope: float = 0.2

    @property
    def shard(self):
        return self.np_ // self.ncores

    @property
    def nblk(self):
        return self.shard // 128

    @property
    def half(self):
        return self.np_ // 2

    @property
    def tpb(self):
        return self.tlo + self.thi


def fold_attn(a, H, C):
    A = np.zeros((H * C, H), np.float32)
    for h in range(H):
        A[h * C:(h + 1) * C, h] = a[h]
    return A


def pack_idx16(idx):
    """int array [n] (n % 128 == 0) -> [128, n//16] int16 dma_gather layout."""
    n = len(idx)
    arr = np.zeros((16, n // 16), dtype=np.int16)
    arr[np.arange(n) % 16, np.arange(n) // 16] = idx
    return np.tile(arr, (8, 1))


def prep_edges(cfg: Cfg, edge_index):
    """Returns srcp/dstp [ncores, nblk, tpb, 128] (int64; dst sentinel 999)
    and sets cfg.tlo/thi."""
    n, npd, sh = cfg.n_real, cfg.np_, cfg.shard
    src = np.concatenate([edge_index[0], np.arange(n)]).astype(np.int64)
    dst = np.concatenate([edge_index[1], np.arange(n)]).astype(np.int64)
    order = np.argsort(dst // sh, kind="stable")
    buckets = [[None] * cfg.nblk for _ in range(cfg.ncores)]
    tlo_b = [1] * cfg.nblk
    thi_b = [1] * cfg.nblk
    for ci in range(cfg.ncores):
        m = (dst // sh) == ci
        s, d = src[m], dst[m] - ci * sh
        for b in range(cfg.nblk):
            mb = (d // 128) == b
            sb, db = s[mb], d[mb] % 128
            lo = sb < cfg.half
            buckets[ci][b] = ((sb[lo], db[lo]), (sb[~lo] - cfg.half, db[~lo]))
            tlo_b[b] = max(tlo_b[b], -(-len(sb[lo]) // 128))
            thi_b[b] = max(thi_b[b], -(-int((~lo).sum()) // 128))
    cfg.tlo, cfg.thi = max(tlo_b), max(thi_b)
    cfg.tlo_b, cfg.thi_b = tuple(tlo_b), tuple(thi_b)
    tlo, thi = cfg.tlo, cfg.thi
    tpb = cfg.tpb
    srcp = np.zeros((cfg.ncores, cfg.nblk, tpb, 128), np.int64)
    dstp = np.full((cfg.ncores, cfg.nblk, tpb, 128), 999, np.int64)
    for ci in range(cfg.ncores):
        for b in range(cfg.nblk):
            (slo, dlo), (shi, dhi) = buckets[ci][b]
            srcp[ci, b, :tlo].flat[: len(slo)] = slo
            dstp[ci, b, :tlo].flat[: len(dlo)] = dlo
            srcp[ci, b, tlo:].flat[: len(shi)] = shi
            dstp[ci, b, tlo:].flat[: len(dhi)] = dhi
    return srcp, dstp


def prep_inputs(cfg: Cfg, inputs):
    """Build per-core in_maps (list of dicts)."""
    H, C, F = cfg.h, cfg.c, cfg.f
    srcp, dstp = prep_edges(cfg, inputs["edge_index"])
    tlo, thi, tpb = cfg.tlo, cfg.thi, cfg.tpb

    xpad = np.zeros((cfg.np_, F), np.float32)
    xpad[: cfg.n_real] = inputs["x"]

    W1e = np.concatenate(
        [inputs["W1"], inputs["W1"] @ fold_attn(inputs["a_src1"], H, C),
         inputs["W1"] @ fold_attn(inputs["a_dst1"], H, C)], axis=1)  # [F,136]
    W2e = np.concatenate(
        [inputs["W2"], inputs["W2"] @ fold_attn(inputs["a_src2"], H, C),
         inputs["W2"] @ fold_attn(inputs["a_dst2"], H, C)], axis=1)

    batch = np.asarray(inputs["batch"]).astype(np.int64)
    batch_pad = np.full(cfg.np_, 999, np.int64)
    batch_pad[: cfg.n_real] = batch
    cnt = np.bincount(batch, minlength=cfg.gp).astype(np.float32)
    rcnt = (1.0 / np.maximum(cnt, 1.0)).astype(np.float32)

    shared = dict(
        w1e=W1e.astype(np.float32), w2e=W2e.astype(np.float32),
        b1bc=np.tile(inputs["b1"][None, :], (128, 1)).astype(np.float32),
        b2bc=np.tile(inputs["b2"][None, :], (128, 1)).astype(np.float32),
        g1row=inputs["g1"][None, :].astype(np.float32),
        be1row=inputs["be1"][None, :].astype(np.float32),
        g2row=inputs["g2"][None, :].astype(np.float32),
        be2row=inputs["be2"][None, :].astype(np.float32),
        fcw=inputs["fcW"].astype(np.float32),
        fcbbc=np.tile(inputs["fcb"][:, None], (1, cfg.gp)).astype(np.float32),
        rcntbc=np.tile(rcnt[None, :], (cfg.k, 1)).astype(np.float32),
        ident=np.eye(128, dtype=np.float32),
        giota=np.tile(np.arange(cfg.gp, dtype=np.float32)[None, :], (128, 1)),
        onescol=np.ones((128, 1), np.float32),
        onesrow=np.ones((1, 128), np.float32),
    )

    in_maps = []
    iota128 = np.arange(128, dtype=np.int64)
    seldt = mybir.dt.np(mybir.dt.float8e4) if cfg.bf16_mm else np.float32
    for ci in range(cfg.ncores):
        idx_lo = np.zeros((cfg.nblk, 128, tlo * 8), np.int16)
        idx_hi = np.zeros((cfg.nblk, 128, thi * 8), np.int16)
        for b in range(cfg.nblk):
            idx_lo[b] = pack_idx16(srcp[ci, b, :tlo].reshape(-1))
            idx_hi[b] = pack_idx16(srcp[ci, b, tlo:].reshape(-1))
        dst_blk = dstp[ci]  # [nblk, tpb, 128] (999 pad)
        # selp[b, p, t, j] = (dst[b, t, p] == j): stationary scatter selector
        selp = (dst_blk.transpose(0, 2, 1)[:, :, :, None]
                == iota128[None, None, None, :]).astype(seldt)
        # selT[b, p, t*128+k] = (dst[b, t, k] == p): arx gather selector
        selT = (dst_blk[:, None, :, :]
                == iota128[None, :, None, None]).astype(seldt)
        selT = selT.reshape(cfg.nblk, 128, tpb * 128)
        sl = slice(ci * cfg.shard, (ci + 1) * cfg.shard)
        nm = np.zeros((128, cfg.nblk), np.float32)
        bc = np.zeros((128, cfg.nblk), np.float32)
        ids = np.arange(ci * cfg.shard, (ci + 1) * cfg.shard)
        nm[:] = (ids.reshape(cfg.nblk, 128).T < cfg.n_real)
        bc[:] = batch_pad[ids].reshape(cfg.nblk, 128).T.astype(np.float32)
        xs = xpad[sl].reshape(cfg.nblk, 128, F).transpose(1, 0, 2)
        in_maps.append(dict(
            x_shard=np.ascontiguousarray(xs).reshape(128, cfg.nblk * F),
            idx_lo=idx_lo, idx_hi=idx_hi,
            selp=selp,
            selT=selT,
            node_mask=nm, batchcol=bc,
            **shared,
        ))
    return in_maps


# ---------------------------------------------------------------------------
# numpy model (for validation at any cfg)
# ---------------------------------------------------------------------------

def numpy_forward(cfg: Cfg, inputs):
    H, C, F = cfg.h, cfg.c, cfg.f
    srcp, dstp = prep_edges(cfg, inputs["edge_index"])
    xpad = np.zeros((cfg.np_, F), np.float32)
    xpad[: cfg.n_real] = inputs["x"]

    def layer(xp, W, asrc, adst, b):
        We = np.concatenate([W, W @ fold_attn(asrc, H, C), W @ fold_attn(adst, H, C)], 1)
        tab = xp @ We
        out = np.zeros((cfg.np_, F), np.float32)
        for ci in range(cfg.ncores):
            for bi in range(cfg.nblk):
                base = ci * cfg.shard + bi * 128
                acc = np.zeros((128, F + H), np.float32)
                ar_blk = tab[base: base + 128, F + H: F + 2 * H]
                for t in range(cfg.tpb):
                    s = srcp[ci, bi, t] + (cfg.half if t >= cfg.tlo else 0)
                    dl = dstp[ci, bi, t]
                    grow = tab[s]
                    sel = (dl[:, None] == np.arange(128)[None, :]).astype(np.float32)
                    e = grow[:, F:F + H] + sel @ ar_blk
                    e = np.where(e > 0, e, cfg.neg_slope * e).astype(np.float32)
                    p = np.exp(e).astype(np.float32)
                    w = grow[:, :F] * np.repeat(p, C, 1)
                    acc += sel.T @ np.concatenate([w, p], 1)
                ssum = np.maximum(np.repeat(acc[:, F:], C, 1), 1e-30)
                out[base:base + 128] = acc[:, :F] / ssum + b
        return out

    def bn_elu(hh, g, be):
        s, ss = hh[:cfg.n_real].sum(0), (hh[:cfg.n_real] ** 2).sum(0)
        mu = s / cfg.n_real
        var = ss / cfg.n_real - mu ** 2
        sc = g / np.sqrt(var + cfg.eps)
        sh = be - mu * sc
        y = hh * sc + sh
        return (np.where(y > 0, y, np.exp(np.minimum(y, 0)) - 1)).astype(np.float32)

    h1 = layer(xpad, inputs["W1"], inputs["a_src1"], inputs["a_dst1"], inputs["b1"])
    h1n = bn_elu(h1, inputs["g1"], inputs["be1"])
    h2 = layer(h1n, inputs["W2"], inputs["a_src2"], inputs["a_dst2"], inputs["b2"])
    h2n = bn_elu(h2, inputs["g2"], inputs["be2"])

    batch = np.asarray(inputs["batch"]).astype(np.int64)
    gsel = np.zeros((cfg.n_real, cfg.gp), np.float32)
    gsel[np.arange(cfg.n_real), batch] = 1.0
    pooled = h2n[:cfg.n_real].T @ gsel
    fc = inputs["fcW"].T.astype(np.float32) @ pooled
    cnt = np.bincount(batch, minlength=cfg.gp).astype(np.float32)
    fc = fc / np.maximum(cnt, 1.0)[None, :] + inputs["fcb"][:, None]
    return fc[:, :cfg.g].T  # [g, k]


# ---------------------------------------------------------------------------
# Bass program
# ---------------------------------------------------------------------------

def build_nc(cfg: Cfg):
    NB, TPB, TLO, THI = cfg.nblk, cfg.tpb, cfg.tlo, cfg.thi
    F, H, C, RF = cfg.f, cfg.h, cfg.c, cfg.rowf
    FH = F + H
    SH = cfg.shard
    GP = cfg.gp

    if cfg.bf16_mm:
        assert cfg.bf16_rows, "bf16_mm requires bf16_rows"
    if cfg.bf16_rows:
        cfg.rowf = 128
        RF = 128
    ALO = 64 if cfg.bf16_rows else F   # f32-slot offset of al in a row
    BF = mybir.dt.bfloat16
    MMDT = BF if cfg.bf16_mm else F32
    SELDT = mybir.dt.float8e4 if cfg.bf16_mm else F32
    nc = bacc.Bacc("TRN2", target_bir_lowering=False, debug=False,
                   num_devices=cfg.ncores, num_swdge_queues=4)

    def ext(name, shape, dtype=F32):
        return nc.dram_tensor(name, shape, dtype, kind="ExternalInput")

    x_shard = ext("x_shard", [128, NB * F])
    idx_lo = ext("idx_lo", [NB, 128, TLO * 8], I16)
    idx_hi = ext("idx_hi", [NB, 128, THI * 8], I16)
    selp_d = ext("selp", [NB, 128, TPB, 128], SELDT)
    selT_d = ext("selT", [NB, 128, TPB * 128], SELDT)
    node_mask = ext("node_mask", [128, NB])
    batchcol = ext("batchcol", [128, NB])
    w1e = ext("w1e", [F, F + 2 * H])
    w2e = ext("w2e", [F, F + 2 * H])
    b1bc = ext("b1bc", [128, F])
    b2bc = ext("b2bc", [128, F])
    g1row = ext("g1row", [1, F])
    be1row = ext("be1row", [1, F])
    g2row = ext("g2row", [1, F])
    be2row = ext("be2row", [1, F])
    fcw = ext("fcw", [F, cfg.k])
    fcbbc = ext("fcbbc", [cfg.k, GP])
    rcntbc = ext("rcntbc", [cfg.k, GP])
    ident_d = ext("ident", [128, 128])
    giota_d = ext("giota", [128, GP])
    onescol_d = ext("onescol", [128, 1])
    onesrow_d = ext("onesrow", [1, 128])

    out_d = nc.dram_tensor("out", [cfg.k, GP], F32, kind="ExternalOutput")

    rg = [list(range(cfg.ncores))]
    shared_as = "Shared" if cfg.ncores > 4 else "Local"

    with tile.TileContext(nc) as tc:
        with (
            tc.tile_pool(name="dram", bufs=1, space="DRAM") as dpool,
            tc.tile_pool(name="persist", bufs=1) as pp,
            tc.tile_pool(name="consts", bufs=1) as cp,
            tc.tile_pool(name="work", bufs=3) as wp_pool,
            tc.tile_pool(name="gath", bufs=4) as gp_pool,
            tc.tile_pool(name="psum", bufs=3, space="PSUM") as ps_pool,
            tc.tile_pool(name="psum1", bufs=1, space="PSUM") as ps1_pool,
        ):
            nc.gpsimd.load_library(library_config.mlp)

            # ---- persistent SBUF ----
            h_cur = pp.tile([128, NB, F], F32)          # shard activations
            ar_sb = pp.tile([128, NB, H], MMDT)
            mask_sb = pp.tile([128, NB], F32)
            bcol_sb = pp.tile([128, NB], F32)

            # ---- constants ----
            w1e_sb = cp.tile([128, F + 2 * H], F32)
            w2e_sb = cp.tile([128, F + 2 * H], F32)
            b1bc_sb = cp.tile([128, F], F32)
            b2bc_sb = cp.tile([128, F], F32)
            ident = cp.tile([128, 128], F32)
            giota = cp.tile([128, GP], F32)
            onescol = cp.tile([128, 1], F32)
            onesrow = cp.tile([1, 128], F32)
            g1_sb = cp.tile([1, F], F32)
            be1_sb = cp.tile([1, F], F32)
            g2_sb = cp.tile([1, F], F32)
            be2_sb = cp.tile([1, F], F32)
            fcw_sb = cp.tile([128, cfg.k], F32)
            fcbbc_sb = cp.tile([cfg.k, GP], F32)
            rcnt_sb = cp.tile([cfg.k, GP], F32)

            for sb, d in [(w1e_sb, w1e), (w2e_sb, w2e), (b1bc_sb, b1bc),
                          (b2bc_sb, b2bc), (ident, ident_d),
                          (giota, giota_d), (onescol, onescol_d),
                          (onesrow, onesrow_d), (g1_sb, g1row), (be1_sb, be1row),
                          (g2_sb, g2row), (be2_sb, be2row), (fcw_sb, fcw),
                          (fcbbc_sb, fcbbc), (rcnt_sb, rcntbc),
                          (mask_sb, node_mask), (bcol_sb, batchcol),
                          (h_cur, x_shard)]:
                nc.sync.dma_start(sb[:], d[:])

            # ---- DRAM internals ----
            ht_in = [dpool.tile([SH, RF], F32, name=f"ht{i}_in") for i in (1, 2)]
            ht = [dpool.tile([cfg.np_, RF], F32, addr_space=shared_as, name=f"ht{i}")
                  for i in (1, 2)]
            bn_in = [dpool.tile([1, 2 * F], F32, name=f"bn{i}_in") for i in (1, 2)]
            bn_out = [dpool.tile([1, 2 * F], F32, addr_space=shared_as, name=f"bn{i}_out")
                      for i in (1, 2)]
            fc_in = dpool.tile([cfg.k, GP], F32)
            fc_out = dpool.tile([cfg.k, GP], F32, addr_space=shared_as)

            # ================= helper phases =================

            def dense_phase(li, wext_sb):
                """h_cur -> table rows (HT_in) + ar_sb; then AllGather."""
                for b in range(NB):
                    tr_ps = ps_pool.tile([128, 128], F32, tag="psA")
                    nc.tensor.transpose(tr_ps[:], h_cur[:, b, :], ident[:])
                    xT = wp_pool.tile([128, 128], F32, tag="xT")
                    nc.vector.tensor_copy(xT[:], tr_ps[:])
                    dp_ps = ps_pool.tile([128, F + 2 * H], F32, tag="psB")
                    nc.tensor.matmul(dp_ps[:], xT[:], wext_sb[:], start=True, stop=True)
                    row = wp_pool.tile([128, RF], F32, tag="row")
                    if cfg.bf16_rows:
                        nc.vector.tensor_copy(
                            row[:, :64].bitcast(mybir.dt.bfloat16), dp_ps[:, :F])
                        nc.vector.tensor_copy(row[:, 64:64 + H], dp_ps[:, F:FH])
                        nc.vector.memset(row[:, 64 + H:], 0.0)
                    else:
                        nc.vector.tensor_copy(row[:, :FH], dp_ps[:, :FH])
                        nc.vector.memset(row[:, FH:], 0.0)
                    nc.vector.tensor_copy(ar_sb[:, b, :], dp_ps[:, FH:FH + H])
                    nc.sync.dma_start(
                        ht_in[li][b * 128:(b + 1) * 128, :],
                        row[:])
                nc.gpsimd.collective_compute(
                    "AllGather", AX.bypass, replica_groups=rg,
                    ins=[ht_in[li][:]], outs=[ht[li][:]])

            def scatter_phase(li, bbc_sb):
                """edge phase: gathers + selector matmuls -> h_cur (+bias).
                Returns the accumulated BN stats PSUM tile."""
                tab = ht[li]
                bn_ps = ps1_pool.tile([1, 2 * F], F32, tag="ps1")
                for b in range(NB):
                    TL, TH = cfg.tlo_b[b], cfg.thi_b[b]
                    ilo_t = gp_pool.tile([128, TLO * 8], I16, tag="ilo")
                    ihi_t = gp_pool.tile([128, THI * 8], I16, tag="ihi")
                    nc.sync.dma_start(ilo_t[:, :TL * 8], idx_lo[b, :, :TL * 8])
                    nc.sync.dma_start(ihi_t[:, :TH * 8], idx_hi[b, :, :TH * 8])
                    glo = gp_pool.tile([128, TLO, RF], F32, tag="glo")
                    ghi = gp_pool.tile([128, THI, RF], F32, tag="ghi")
                    # chunked queue-parallel gathers: lo on queues 0..1, hi 2..3
                    def gather_chunks(dst_t, src_ap, idxs_t, T, qbase):
                        nch = min(cfg.gather_chunks, T)
                        bounds = [T * i // nch for i in range(nch + 1)]
                        for c in range(nch):
                            a, z = bounds[c], bounds[c + 1]
                            if z <= a:
                                continue
                            nc.gpsimd.dma_gather(
                                out_ap=dst_t[:, a:z, :], in_ap=src_ap,
                                idxs_ap=idxs_t[:, a * 8:z * 8],
                                num_idxs=(z - a) * 128,
                                num_idxs_reg=(z - a) * 128, elem_size=RF,
                                queue_num=qbase + c,
                                single_packet=cfg.single_packet)
                    gather_chunks(glo, tab[:cfg.half, :], ilo_t, TL, 0)
                    gather_chunks(ghi, tab[cfg.half:, :], ihi_t, TH, 2)
                    # selectors (host-precomputed, DMA-streamed, fp8)
                    sel = gp_pool.tile([128, TPB, 128], SELDT, tag="sel")
                    selT = gp_pool.tile([128, TPB * 128], SELDT, tag="selT")
                    nc.sync.dma_start(sel[:, :TL, :], selp_d[b, :, :TL, :])
                    nc.sync.dma_start(sel[:, TLO:TLO + TH, :],
                                      selp_d[b, :, TLO:TLO + TH, :])
                    nc.sync.dma_start(selT[:, :TL * 128], selT_d[b, :, :TL * 128])
                    nc.sync.dma_start(selT[:, TLO * 128:(TLO + TH) * 128],
                                      selT_d[b, :, TLO * 128:(TLO + TH) * 128])
                    # ar expand per tile
                    arx_ps = ps_pool.tile([128, TPB, H], F32, tag="psA")
                    real_ts = list(range(TL)) + list(range(TLO, TLO + TH))
                    for t in real_ts:
                        nc.tensor.matmul(arx_ps[:, t, :],
                                         selT[:, t * 128:(t + 1) * 128],
                                         ar_sb[:, b, :], start=True, stop=True)
                    # e = al + ar ; lrelu = max(e, 0.2e); exp -> p
                    wpt = wp_pool.tile([128, TPB, FH], MMDT, tag="wpt")
                    e_sb = wp_pool.tile([128, TPB, H], F32, tag="e")
                    nc.vector.tensor_tensor(e_sb[:, :TL, :],
                                            glo[:, :TL, ALO:ALO + H],
                                            arx_ps[:, :TL, :], AX.add)
                    nc.vector.tensor_tensor(e_sb[:, TLO:TLO + TH, :],
                                            ghi[:, :TH, ALO:ALO + H],
                                            arx_ps[:, TLO:TLO + TH, :], AX.add)
                    eneg = wp_pool.tile([128, TPB, H], F32, tag="eneg")
                    for r0, r1 in ((0, TL), (TLO, TLO + TH)):
                        nc.vector.tensor_scalar(eneg[:, r0:r1, :], e_sb[:, r0:r1, :],
                                                cfg.neg_slope, None, AX.mult)
                        nc.vector.tensor_tensor(e_sb[:, r0:r1, :], e_sb[:, r0:r1, :],
                                                eneg[:, r0:r1, :], AX.max)
                    nc.scalar.activation(wpt[:, :TL, F:FH], e_sb[:, :TL, :], AF.Exp)
                    nc.scalar.activation(wpt[:, TLO:TLO + TH, F:FH],
                                         e_sb[:, TLO:TLO + TH, :], AF.Exp)
                    # w = h * p_expand
                    if cfg.bf16_rows:
                        glo_h = glo[:, :, :64].bitcast(mybir.dt.bfloat16)
                        ghi_h = ghi[:, :, :64].bitcast(mybir.dt.bfloat16)
                    else:
                        glo_h = glo[:, :, :F]
                        ghi_h = ghi[:, :, :F]
                    nc.vector.tensor_tensor(
                        wpt[:, :TL, :F].rearrange("p t (h c) -> p t h c", c=C),
                        glo_h[:, :TL, :].rearrange("p t (h c) -> p t h c", c=C),
                        wpt[:, :TL, F:FH].unsqueeze(3).broadcast_to([128, TL, H, C]),
                        AX.mult)
                    nc.vector.tensor_tensor(
                        wpt[:, TLO:TLO + TH, :F].rearrange("p t (h c) -> p t h c", c=C),
                        ghi_h[:, :TH, :].rearrange("p t (h c) -> p t h c", c=C),
                        wpt[:, TLO:TLO + TH, F:FH].unsqueeze(3).broadcast_to(
                            [128, TH, H, C]),
                        AX.mult)
                    # scatter matmuls
                    acc_ps = ps_pool.tile([128, FH], F32, tag="psB")
                    for i, t in enumerate(real_ts):
                        nc.tensor.matmul(acc_ps[:], sel[:, t, :], wpt[:, t, :],
                                         start=(i == 0),
                                         stop=(i == len(real_ts) - 1))
                    # divide + bias -> h_cur
                    s_sb = wp_pool.tile([128, H], F32, tag="s")
                    nc.vector.tensor_scalar(s_sb[:], acc_ps[:, F:FH], 1e-30, None,
                                            AX.max)
                    r_sb = wp_pool.tile([128, H], F32, tag="r")
                    nc.vector.reciprocal(r_sb[:], s_sb[:])
                    nc.vector.tensor_tensor(
                        h_cur[:, b, :].rearrange("p (h c) -> p h c", c=C),
                        acc_ps[:, :F].rearrange("p (h c) -> p h c", c=C),
                        r_sb[:].unsqueeze(2).broadcast_to([128, H, C]),
                        AX.mult)
                    nc.vector.tensor_tensor(h_cur[:, b, :], h_cur[:, b, :],
                                            bbc_sb[:], AX.add)
                    # fused BN stats accumulation (sum, sum-of-squares)
                    rhs = wp_pool.tile([128, 2 * F], F32, tag="bnrhs")
                    nc.vector.tensor_scalar(rhs[:, :F], h_cur[:, b, :],
                                            mask_sb[:, b].unsqueeze(1), None, AX.mult)
                    nc.scalar.activation(rhs[:, F:], rhs[:, :F], AF.Square)
                    nc.tensor.matmul(bn_ps[:], onescol[:], rhs[:],
                                     start=(b == 0), stop=(b == NB - 1))
                return bn_ps

            def bn_elu_phase(li, g_sb, be_sb, bn_ps, pool_ps=None):
                bn_sb = wp_pool.tile([1, 2 * F], F32, tag="bnrow")
                nc.vector.tensor_copy(bn_sb[:], bn_ps[:])
                nc.sync.dma_start(bn_in[li][:], bn_sb[:])
                nc.gpsimd.collective_compute(
                    "AllReduce", AX.add, replica_groups=rg,
                    ins=[bn_in[li][:]], outs=[bn_out[li][:]])
                st = wp_pool.tile([1, 2 * F], F32, tag="bnst")
                nc.sync.dma_start(st[:], bn_out[li][:])
                # mu = s/n ; var = ss/n - mu^2
                mu = wp_pool.tile([1, F], F32, tag="mu")
                nc.vector.tensor_scalar(mu[:], st[:, :F], 1.0 / cfg.n_real, None,
                                        AX.mult)
                var = wp_pool.tile([1, F], F32, tag="var")
                nc.vector.tensor_scalar(var[:], st[:, F:], 1.0 / cfg.n_real, None,
                                        AX.mult)
                mu2 = wp_pool.tile([1, F], F32, tag="mu2")
                nc.scalar.activation(mu2[:], mu[:], AF.Square)
                nc.vector.tensor_tensor(var[:], var[:], mu2[:], AX.subtract)
                # rstd = 1/sqrt(var+eps)
                nc.vector.tensor_scalar(var[:], var[:], cfg.eps, None, AX.add)
                sd = wp_pool.tile([1, F], F32, tag="sd")
                nc.scalar.activation(sd[:], var[:], AF.Sqrt)
                rstd = wp_pool.tile([1, F], F32, tag="rstd")
                nc.vector.reciprocal(rstd[:], sd[:])
                # scale = g*rstd ; shift = be - mu*scale
                ssrow = wp_pool.tile([1, 2 * F], F32, tag="ssrow")
                nc.vector.tensor_tensor(ssrow[:, :F], g_sb[:], rstd[:], AX.mult)
                musc = wp_pool.tile([1, F], F32, tag="musc")
                nc.vector.tensor_tensor(musc[:], mu[:], ssrow[:, :F], AX.mult)
                nc.vector.tensor_tensor(ssrow[:, F:], be_sb[:], musc[:], AX.subtract)
                # broadcast via K=1 matmul
                bc_ps = ps1_pool.tile([128, 2 * F], F32, tag="ps1")
                nc.tensor.matmul(bc_ps[:], onesrow[:], ssrow[:], start=True, stop=True)
                bc_sb = wp_pool.tile([128, 2 * F], F32, tag="bnbcsb")
                nc.vector.tensor_copy(bc_sb[:], bc_ps[:])
                # normalize + elu
                for b in range(NB):
                    nc.vector.tensor_tensor(h_cur[:, b, :], h_cur[:, b, :],
                                            bc_sb[:, :F], AX.mult)
                    nc.vector.tensor_tensor(h_cur[:, b, :], h_cur[:, b, :],
                                            bc_sb[:, F:], AX.add)
                    neg = wp_pool.tile([128, F], F32, tag="neg")
                    nc.vector.tensor_scalar(neg[:], h_cur[:, b, :], 0.0, None, AX.min)
                    ex = wp_pool.tile([128, F], F32, tag="ex")
                    nc.scalar.activation(ex[:], neg[:], AF.Exp)
                    nc.vector.tensor_scalar(h_cur[:, b, :], h_cur[:, b, :], 0.0, None,
                                            AX.max)
                    nc.vector.tensor_tensor(h_cur[:, b, :], h_cur[:, b, :], ex[:],
                                            AX.add)
                    nc.vector.tensor_scalar(h_cur[:, b, :], h_cur[:, b, :], -1.0,
                                            None, AX.add)
                    if pool_ps is not None:
                        # fused global-mean-pool accumulation
                        gsel = wp_pool.tile([128, GP], F32, tag="gsel")
                        nc.vector.tensor_scalar(gsel[:], giota[:],
                                                bcol_sb[:, b].unsqueeze(1), None,
                                                AX.is_equal)
                        nc.tensor.matmul(pool_ps[:], h_cur[:, b, :], gsel[:],
                                         start=(b == 0), stop=(b == NB - 1))

            # ================= program =================
            for _rep in range(cfg.repeat):
              dense_phase(0, w1e_sb)
              bn1_ps = scatter_phase(0, b1bc_sb)
              bn_elu_phase(0, g1_sb, be1_sb, bn1_ps)
              dense_phase(1, w2e_sb)
              bn2_ps = scatter_phase(1, b2bc_sb)
              pool_ps = ps1_pool.tile([128, GP], F32, tag="ps2")
              bn_elu_phase(1, g2_sb, be2_sb, bn2_ps, pool_ps=pool_ps)
            # FC head
            try:
                pool_sb = wp_pool.tile([128, GP], F32, tag="poolsb")
                nc.vector.tensor_copy(pool_sb[:], pool_ps[:])
                fc_ps = ps1_pool.tile([cfg.k, GP], F32, tag="ps1")
                nc.tensor.matmul(fc_ps[:], fcw_sb[:], pool_sb[:], start=True, stop=True)
                fc_sb = wp_pool.tile([cfg.k, GP], F32, tag="fcsb")
                nc.vector.tensor_copy(fc_sb[:], fc_ps[:])
                nc.sync.dma_start(fc_in[:], fc_sb[:])
                nc.gpsimd.collective_compute("AllReduce", AX.add, replica_groups=rg,
                                             ins=[fc_in[:]], outs=[fc_out[:]])
                fin = wp_pool.tile([cfg.k, GP], F32, tag="fin")
                nc.sync.dma_start(fin[:], fc_out[:])
                nc.vector.tensor_tensor(fin[:], fin[:], rcnt_sb[:], AX.mult)
                nc.vector.tensor_tensor(fin[:], fin[:], fcbbc_sb[:], AX.add)
                nc.sync.dma_start(out_d[:], fin[:])
            except _SkipRest:
                pass

    nc.compile()
    return nc


# ---------------------------------------------------------------------------
# harness entry point: full inputs in, full output out
# ---------------------------------------------------------------------------

_NC_CACHE = {}


def kernel(**inputs):
    """Full-input GAT forward on 8 NeuronCores. Returns [500, 6] float32."""
    from concourse.bass_utils import run_bass_kernel_spmd

    cfg = Cfg(bf16_rows=True, bf16_mm=True)
    in_maps = prep_inputs(cfg, inputs)
    key = (cfg.tlo, cfg.thi, cfg.bf16_rows, cfg.bf16_mm)
    if key not in _NC_CACHE:
        _NC_CACHE[key] = build_nc(cfg)
    nc = _NC_CACHE[key]
    res = run_bass_kernel_spmd(nc, in_maps, core_ids=list(range(cfg.ncores)))
    out = res.results[0]["out"]
    return np.ascontiguousarray(out[:, :cfg.g].T).astype(np.float32)



# revision 37
# speedup vs baseline: 1.5877x; 1.2542x over previous
"""GAT kernel for TRN2: host prep + Bass program builder + numpy model.

Sharding: nodes (and their in-edges) partitioned across cores by contiguous
shard; per dst-block-of-128 selector-matmul scatter; edge gathers of packed
table rows [h(128f32) | al(4f32) | pad] = 768B via gpsimd dma_gather with a
lo/hi table split (int16 index limit); inter-layer AllGather of the table;
BN via AllReduce of partial sums; pooling via transposed graph-selector
matmul; tiny FC + final AllReduce.
"""
from dataclasses import dataclass

import numpy as np

import concourse.bacc as bacc
import concourse.bass as bass
import concourse.mybir as mybir
import concourse.tile as tile
from concourse import library_config

F32 = mybir.dt.float32
I16 = mybir.dt.int16
I8 = mybir.dt.int8
AX = mybir.AluOpType
AF = mybir.ActivationFunctionType


class _SkipRest(Exception):
    pass



@dataclass
class Cfg:
    ncores: int = 8
    n_real: int = 50000       # real nodes
    np_: int = 50176          # padded nodes (multiple of ncores*128)
    e_raw: int = 800000       # edges before self loops
    g: int = 500              # graphs
    gp: int = 512             # padded graphs (pool matmul free dim)
    tlo: int = 0              # max tiles per block from lo table (computed in prep)
    thi: int = 0
    tlo_b: tuple = ()         # per-block lo tile counts (max over cores)
    thi_b: tuple = ()
    f: int = 128              # features (in = out = 128)
    h: int = 4
    c: int = 32
    k: int = 6
    eps: float = 1e-5
    rowf: int = 192           # table row floats (768B); 128 when bf16_rows
    stage: int = 9            # debug: how much of the program to emit
    repeat: int = 1           # timing: execute the whole body N times
    bf16_rows: bool = False   # pack h as bf16 in 512B table rows
    bf16_mm: bool = False     # bf16 selectors + weighted features (FWL matmuls)
    single_packet: bool = True  # coalesce gather descriptor stream packets
    gather_chunks: int = 2    # split each lo/hi gather into N queue-parallel chunks
    neg_slope: float = 0.2

    @property
    def shard(self):
        return self.np_ // self.ncores

    @property
    def nblk(self):
        return self.shard // 128

    @property
    def arows(self):
        return (self.nblk // 2) * 128      # 3072: shard rows in table A

    @property
    def brows(self):
        return self.shard - self.arows     # 3200: shard rows in table B

    @property
    def tpb(self):
        return self.tlo + self.thi


def fold_attn(a, H, C):
    A = np.zeros((H * C, H), np.float32)
    for h in range(H):
        A[h * C:(h + 1) * C, h] = a[h]
    return A


def pack_idx16(idx):
    """int array [n] (n % 128 == 0) -> [128, n//16] int16 dma_gather layout."""
    n = len(idx)
    arr = np.zeros((16, n // 16), dtype=np.int16)
    arr[np.arange(n) % 16, np.arange(n) // 16] = idx
    return np.tile(arr, (8, 1))


def prep_edges(cfg: Cfg, edge_index):
    """Returns srcp/dstp [ncores, nblk, tpb, 128] (int64; dst sentinel 999)
    and sets cfg.tlo/thi. Srcs are split into table A (shard offset < arows)
    and table B, matching the split AllGather layout."""
    n, npd, sh = cfg.n_real, cfg.np_, cfg.shard
    AR, BR = cfg.arows, cfg.brows
    src = np.concatenate([edge_index[0], np.arange(n)]).astype(np.int64)
    dst = np.concatenate([edge_index[1], np.arange(n)]).astype(np.int64)
    buckets = [[None] * cfg.nblk for _ in range(cfg.ncores)]
    tlo_b = [1] * cfg.nblk
    thi_b = [1] * cfg.nblk
    for ci in range(cfg.ncores):
        m = (dst // sh) == ci
        s, d = src[m], dst[m] - ci * sh
        for b in range(cfg.nblk):
            mb = (d // 128) == b
            sb, db = s[mb], d[mb] % 128
            core_s, off = sb // sh, sb % sh
            lo = off < AR
            rowa = core_s[lo] * AR + off[lo]
            rowb = core_s[~lo] * BR + (off[~lo] - AR)
            buckets[ci][b] = ((rowa, db[lo]), (rowb, db[~lo]))
            tlo_b[b] = max(tlo_b[b], -(-len(rowa) // 128))
            thi_b[b] = max(thi_b[b], -(-len(rowb) // 128))
    cfg.tlo, cfg.thi = max(tlo_b), max(thi_b)
    cfg.tlo_b, cfg.thi_b = tuple(tlo_b), tuple(thi_b)
    tlo, thi = cfg.tlo, cfg.thi
    tpb = cfg.tpb
    srcp = np.zeros((cfg.ncores, cfg.nblk, tpb, 128), np.int64)
    dstp = np.full((cfg.ncores, cfg.nblk, tpb, 128), 999, np.int64)
    for ci in range(cfg.ncores):
        for b in range(cfg.nblk):
            (slo, dlo), (shi, dhi) = buckets[ci][b]
            srcp[ci, b, :tlo].flat[: len(slo)] = slo
            dstp[ci, b, :tlo].flat[: len(dlo)] = dlo
            srcp[ci, b, tlo:].flat[: len(shi)] = shi
            dstp[ci, b, tlo:].flat[: len(dhi)] = dhi
    return srcp, dstp


def prep_inputs(cfg: Cfg, inputs):
    """Build per-core in_maps (list of dicts)."""
    H, C, F = cfg.h, cfg.c, cfg.f
    srcp, dstp = prep_edges(cfg, inputs["edge_index"])
    tlo, thi, tpb = cfg.tlo, cfg.thi, cfg.tpb

    xpad = np.zeros((cfg.np_, F), np.float32)
    xpad[: cfg.n_real] = inputs["x"]

    bf16 = mybir.dt.np(mybir.dt.bfloat16)
    W1e = np.concatenate(
        [inputs["W1"], inputs["W1"] @ fold_attn(inputs["a_dst1"], H, C)], axis=1)
    W2e = np.concatenate(
        [inputs["W2"], inputs["W2"] @ fold_attn(inputs["a_dst2"], H, C)], axis=1)

    batch = np.asarray(inputs["batch"]).astype(np.int64)
    batch_pad = np.full(cfg.np_, 999, np.int64)
    batch_pad[: cfg.n_real] = batch
    cnt = np.bincount(batch, minlength=cfg.gp).astype(np.float32)
    rcnt = (1.0 / np.maximum(cnt, 1.0)).astype(np.float32)

    shared = dict(
        w1e=W1e.astype(np.float32), w2e=W2e.astype(np.float32),
        b1bc=np.tile(inputs["b1"][None, :], (128, 1)).astype(np.float32),
        b2bc=np.tile(inputs["b2"][None, :], (128, 1)).astype(np.float32),
        g1row=inputs["g1"][None, :].astype(np.float32),
        be1row=inputs["be1"][None, :].astype(np.float32),
        g2row=inputs["g2"][None, :].astype(np.float32),
        be2row=inputs["be2"][None, :].astype(np.float32),
        fcw=inputs["fcW"].astype(np.float32),
        fcbbc=np.tile(inputs["fcb"][:, None], (1, cfg.gp)).astype(np.float32),
        rcntbc=np.tile(rcnt[None, :], (cfg.k, 1)).astype(np.float32),
        ident=np.eye(128, dtype=np.float32),
        avec1=np.tile(np.asarray(inputs["a_src1"]).reshape(1, -1),
                      (128, 1)).astype(bf16),
        avec2=np.tile(np.asarray(inputs["a_src2"]).reshape(1, -1),
                      (128, 1)).astype(bf16),
        onescol=np.ones((128, 1), np.float32),
        onesrow=np.ones((1, 128), np.float32),
    )

    in_maps = []
    iota128 = np.arange(128, dtype=np.int64)
    seldt = mybir.dt.np(mybir.dt.float8e4) if cfg.bf16_mm else np.float32
    for ci in range(cfg.ncores):
        idx_lo = np.zeros((cfg.nblk, 128, tlo * 8), np.int16)
        idx_hi = np.zeros((cfg.nblk, 128, thi * 8), np.int16)
        for b in range(cfg.nblk):
            idx_lo[b] = pack_idx16(srcp[ci, b, :tlo].reshape(-1))
            idx_hi[b] = pack_idx16(srcp[ci, b, tlo:].reshape(-1))
        dst_blk = dstp[ci]  # [nblk, tpb, 128] (999 pad)
        # selp[b, p, t, j] = (dst[b, t, p] == j): stationary scatter selector
        selp = (dst_blk.transpose(0, 2, 1)[:, :, :, None]
                == iota128[None, None, None, :]).astype(seldt)
        # selT[b, p, t*128+k] = (dst[b, t, k] == p): arx gather selector
        selT = (dst_blk[:, None, :, :]
                == iota128[None, :, None, None]).astype(seldt)
        selT = selT.reshape(cfg.nblk, 128, tpb * 128)
        sl = slice(ci * cfg.shard, (ci + 1) * cfg.shard)
        nm = np.zeros((128, cfg.nblk), np.float32)
        bc = np.zeros((128, cfg.nblk), np.float32)
        ids = np.arange(ci * cfg.shard, (ci + 1) * cfg.shard)
        nm[:] = (ids.reshape(cfg.nblk, 128).T < cfg.n_real)
        bc[:] = batch_pad[ids].reshape(cfg.nblk, 128).T.astype(np.float32)
        # host-precomputed pool selector [NB, 128, GP] (999 pad -> all-zero)
        gselp = (bc.T[:, :, None]
                 == np.arange(cfg.gp, dtype=np.float32)[None, None, :]).astype(bf16)
        xs = xpad[sl].reshape(cfg.nblk, 128, F).transpose(1, 0, 2)
        in_maps.append(dict(
            x_shard=np.ascontiguousarray(xs).reshape(128, cfg.nblk * F),
            idx_lo=idx_lo, idx_hi=idx_hi,
            selp=selp,
            selT=selT,
            gselp=gselp,
            node_mask=nm, batchcol=bc,
            **shared,
        ))
    return in_maps


# ---------------------------------------------------------------------------
# numpy model (for validation at any cfg)
# ---------------------------------------------------------------------------

def numpy_forward(cfg: Cfg, inputs):
    H, C, F = cfg.h, cfg.c, cfg.f
    srcp, dstp = prep_edges(cfg, inputs["edge_index"])
    xpad = np.zeros((cfg.np_, F), np.float32)
    xpad[: cfg.n_real] = inputs["x"]

    def layer(xp, W, asrc, adst, b):
        We = np.concatenate([W, W @ fold_attn(asrc, H, C), W @ fold_attn(adst, H, C)], 1)
        tab = xp @ We
        out = np.zeros((cfg.np_, F), np.float32)
        for ci in range(cfg.ncores):
            for bi in range(cfg.nblk):
                base = ci * cfg.shard + bi * 128
                acc = np.zeros((128, F + H), np.float32)
                ar_blk = tab[base: base + 128, F + H: F + 2 * H]
                for t in range(cfg.tpb):
                    s = srcp[ci, bi, t] + (cfg.half if t >= cfg.tlo else 0)
                    dl = dstp[ci, bi, t]
                    grow = tab[s]
                    sel = (dl[:, None] == np.arange(128)[None, :]).astype(np.float32)
                    e = grow[:, F:F + H] + sel @ ar_blk
                    e = np.where(e > 0, e, cfg.neg_slope * e).astype(np.float32)
                    p = np.exp(e).astype(np.float32)
                    w = grow[:, :F] * np.repeat(p, C, 1)
                    acc += sel.T @ np.concatenate([w, p], 1)
                ssum = np.maximum(np.repeat(acc[:, F:], C, 1), 1e-30)
                out[base:base + 128] = acc[:, :F] / ssum + b
        return out

    def bn_elu(hh, g, be):
        s, ss = hh[:cfg.n_real].sum(0), (hh[:cfg.n_real] ** 2).sum(0)
        mu = s / cfg.n_real
        var = ss / cfg.n_real - mu ** 2
        sc = g / np.sqrt(var + cfg.eps)
        sh = be - mu * sc
        y = hh * sc + sh
        return (np.where(y > 0, y, np.exp(np.minimum(y, 0)) - 1)).astype(np.float32)

    h1 = layer(xpad, inputs["W1"], inputs["a_src1"], inputs["a_dst1"], inputs["b1"])
    h1n = bn_elu(h1, inputs["g1"], inputs["be1"])
    h2 = layer(h1n, inputs["W2"], inputs["a_src2"], inputs["a_dst2"], inputs["b2"])
    h2n = bn_elu(h2, inputs["g2"], inputs["be2"])

    batch = np.asarray(inputs["batch"]).astype(np.int64)
    gsel = np.zeros((cfg.n_real, cfg.gp), np.float32)
    gsel[np.arange(cfg.n_real), batch] = 1.0
    pooled = h2n[:cfg.n_real].T @ gsel
    fc = inputs["fcW"].T.astype(np.float32) @ pooled
    cnt = np.bincount(batch, minlength=cfg.gp).astype(np.float32)
    fc = fc / np.maximum(cnt, 1.0)[None, :] + inputs["fcb"][:, None]
    return fc[:, :cfg.g].T  # [g, k]


# ---------------------------------------------------------------------------
# Bass program
# ---------------------------------------------------------------------------

def build_nc(cfg: Cfg):
    NB, TPB, TLO, THI = cfg.nblk, cfg.tpb, cfg.tlo, cfg.thi
    F, H, C, RF = cfg.f, cfg.h, cfg.c, cfg.rowf
    FH = F + H
    SH = cfg.shard
    GP = cfg.gp

    if cfg.bf16_mm:
        assert cfg.bf16_rows, "bf16_mm requires bf16_rows"
    if cfg.bf16_rows:
        cfg.rowf = 64              # 256B rows: h in bf16 only
        RF = 64
    BF = mybir.dt.bfloat16
    MMDT = BF if cfg.bf16_mm else F32
    SELDT = mybir.dt.float8e4 if cfg.bf16_mm else F32
    nc = bacc.Bacc("TRN2", target_bir_lowering=False, debug=False,
                   num_devices=cfg.ncores, num_swdge_queues=4)

    def ext(name, shape, dtype=F32):
        return nc.dram_tensor(name, shape, dtype, kind="ExternalInput")

    x_shard = ext("x_shard", [128, NB * F])
    idx_lo = ext("idx_lo", [NB, 128, TLO * 8], I16)
    idx_hi = ext("idx_hi", [NB, 128, THI * 8], I16)
    selp_d = ext("selp", [NB, 128, TPB, 128], SELDT)
    selT_d = ext("selT", [NB, 128, TPB * 128], SELDT)
    node_mask = ext("node_mask", [128, NB])
    batchcol = ext("batchcol", [128, NB])
    gselp_d = ext("gselp", [NB, 128, GP], BF)
    w1e = ext("w1e", [F, F + H])
    w2e = ext("w2e", [F, F + H])
    avec1_d = ext("avec1", [128, F], BF)
    avec2_d = ext("avec2", [128, F], BF)
    b1bc = ext("b1bc", [128, F])
    b2bc = ext("b2bc", [128, F])
    g1row = ext("g1row", [1, F])
    be1row = ext("be1row", [1, F])
    g2row = ext("g2row", [1, F])
    be2row = ext("be2row", [1, F])
    fcw = ext("fcw", [F, cfg.k])
    fcbbc = ext("fcbbc", [cfg.k, GP])
    rcntbc = ext("rcntbc", [cfg.k, GP])
    ident_d = ext("ident", [128, 128])
    onescol_d = ext("onescol", [128, 1])
    onesrow_d = ext("onesrow", [1, 128])

    out_d = nc.dram_tensor("out", [cfg.k, GP], F32, kind="ExternalOutput")

    rg = [list(range(cfg.ncores))]
    shared_as = "Shared" if cfg.ncores > 4 else "Local"

    with tile.TileContext(nc) as tc:
        with (
            tc.tile_pool(name="dram", bufs=1, space="DRAM") as dpool,
            tc.tile_pool(name="persist", bufs=1) as pp,
            tc.tile_pool(name="consts", bufs=1) as cp,
            tc.tile_pool(name="work", bufs=3) as wp_pool,
            tc.tile_pool(name="gath", bufs=4) as gp_pool,
            tc.tile_pool(name="psum", bufs=3, space="PSUM") as ps_pool,
            tc.tile_pool(name="psum1", bufs=1, space="PSUM") as ps1_pool,
        ):
            nc.gpsimd.load_library(library_config.mlp)

            # ---- persistent SBUF ----
            h_cur = pp.tile([128, NB, F], F32)          # shard activations
            ar_sb = pp.tile([128, NB, H], MMDT)
            mask_sb = pp.tile([128, NB], F32)
            bcol_sb = pp.tile([128, NB], F32)

            # ---- constants ----
            w1e_sb = cp.tile([128, F + H], F32)
            w2e_sb = cp.tile([128, F + H], F32)
            b1bc_sb = cp.tile([128, F], F32)
            b2bc_sb = cp.tile([128, F], F32)
            ident = cp.tile([128, 128], F32)
            avec1_sb = cp.tile([128, F], BF)
            avec2_sb = cp.tile([128, F], BF)
            onescol = cp.tile([128, 1], F32)
            onesrow = cp.tile([1, 128], F32)
            g1_sb = cp.tile([1, F], F32)
            be1_sb = cp.tile([1, F], F32)
            g2_sb = cp.tile([1, F], F32)
            be2_sb = cp.tile([1, F], F32)
            fcw_sb = cp.tile([128, cfg.k], F32)
            fcbbc_sb = cp.tile([cfg.k, GP], F32)
            rcnt_sb = cp.tile([cfg.k, GP], F32)

            for sb, d in [(w1e_sb, w1e), (w2e_sb, w2e), (b1bc_sb, b1bc),
                          (b2bc_sb, b2bc), (ident, ident_d),
                          (avec1_sb, avec1_d), (avec2_sb, avec2_d),
                          (onescol, onescol_d),
                          (onesrow, onesrow_d), (g1_sb, g1row), (be1_sb, be1row),
                          (g2_sb, g2row), (be2_sb, be2row), (fcw_sb, fcw),
                          (fcbbc_sb, fcbbc), (rcnt_sb, rcntbc),
                          (mask_sb, node_mask), (bcol_sb, batchcol),
                          (h_cur, x_shard)]:
                nc.sync.dma_start(sb[:], d[:])

            # ---- DRAM internals ----
            AR, BR = cfg.arows, cfg.brows
            ht_in = [dpool.tile([SH, RF], F32, name=f"ht{i}_in") for i in (1, 2)]
            ht_a = [dpool.tile([cfg.ncores * AR, RF], F32, addr_space=shared_as,
                               name=f"ht{i}a") for i in (1, 2)]
            ht_b = [dpool.tile([cfg.ncores * BR, RF], F32, addr_space=shared_as,
                               name=f"ht{i}b") for i in (1, 2)]
            bn_in = [dpool.tile([1, 2 * F], F32, name=f"bn{i}_in") for i in (1, 2)]
            bn_out = [dpool.tile([1, 2 * F], F32, addr_space=shared_as, name=f"bn{i}_out")
                      for i in (1, 2)]
            fc_in = dpool.tile([cfg.k, GP], F32)
            fc_out = dpool.tile([cfg.k, GP], F32, addr_space=shared_as)

            # ================= helper phases =================

            def dense_phase(li, wext_sb):
                """h_cur -> table rows (HT_in) + ar_sb; split AllGather A/B."""
                nba = AR // 128
                for b in range(NB):
                    tr_ps = ps_pool.tile([128, 128], F32, tag="psA")
                    nc.tensor.transpose(tr_ps[:], h_cur[:, b, :], ident[:])
                    xT = wp_pool.tile([128, 128], F32, tag="xT")
                    nc.scalar.activation(xT[:], tr_ps[:], AF.Copy)
                    dp_ps = ps_pool.tile([128, F + H], F32, tag="psB")
                    nc.tensor.matmul(dp_ps[:], xT[:], wext_sb[:], start=True, stop=True)
                    row = wp_pool.tile([128, RF], F32, tag="row")
                    nc.scalar.activation(
                        row[:, :RF].bitcast(mybir.dt.bfloat16), dp_ps[:, :F],
                        AF.Copy)
                    nc.vector.tensor_copy(ar_sb[:, b, :], dp_ps[:, F:F + H])
                    nc.sync.dma_start(
                        ht_in[li][b * 128:(b + 1) * 128, :],
                        row[:])
                    if b == nba - 1:
                        nc.gpsimd.collective_compute(
                            "AllGather", AX.bypass, replica_groups=rg,
                            ins=[ht_in[li][:AR, :]], outs=[ht_a[li][:]])
                nc.gpsimd.collective_compute(
                    "AllGather", AX.bypass, replica_groups=rg,
                    ins=[ht_in[li][AR:, :]], outs=[ht_b[li][:]])

            def scatter_phase(li, bbc_sb, avec_sb):
                """edge phase: gathers + selector matmuls -> h_cur (+bias).
                Returns the accumulated BN stats PSUM tile."""
                bn_ps = ps1_pool.tile([1, 2 * F], F32, tag="ps1")
                for b in range(NB):
                    TL, TH = cfg.tlo_b[b], cfg.thi_b[b]
                    ilo_t = gp_pool.tile([128, TLO * 8], I16, tag="ilo")
                    ihi_t = gp_pool.tile([128, THI * 8], I16, tag="ihi")
                    nc.sync.dma_start(ilo_t[:, :TL * 8], idx_lo[b, :, :TL * 8])
                    nc.sync.dma_start(ihi_t[:, :TH * 8], idx_hi[b, :, :TH * 8])
                    glo = gp_pool.tile([128, TLO, RF], F32, tag="glo")
                    ghi = gp_pool.tile([128, THI, RF], F32, tag="ghi")
                    # chunked queue-parallel gathers: lo on queues 0..1, hi 2..3
                    def gather_chunks(dst_t, src_ap, idxs_t, T, qbase):
                        nch = min(cfg.gather_chunks, T)
                        bounds = [T * i // nch for i in range(nch + 1)]
                        for c in range(nch):
                            a, z = bounds[c], bounds[c + 1]
                            if z <= a:
                                continue
                            nc.gpsimd.dma_gather(
                                out_ap=dst_t[:, a:z, :], in_ap=src_ap,
                                idxs_ap=idxs_t[:, a * 8:z * 8],
                                num_idxs=(z - a) * 128,
                                num_idxs_reg=(z - a) * 128, elem_size=RF,
                                queue_num=qbase + c,
                                single_packet=cfg.single_packet)
                    gather_chunks(glo, ht_a[li][:], ilo_t, TL, 0)
                    gather_chunks(ghi, ht_b[li][:], ihi_t, TH, 2)
                    # selectors (host-precomputed, DMA-streamed, fp8)
                    sel = gp_pool.tile([128, TPB, 128], SELDT, tag="sel")
                    selT = gp_pool.tile([128, TPB * 128], SELDT, tag="selT")
                    nc.sync.dma_start(sel[:, :TL, :], selp_d[b, :, :TL, :])
                    nc.sync.dma_start(sel[:, TLO:TLO + TH, :],
                                      selp_d[b, :, TLO:TLO + TH, :])
                    nc.sync.dma_start(selT[:, :TL * 128], selT_d[b, :, :TL * 128])
                    nc.sync.dma_start(selT[:, TLO * 128:(TLO + TH) * 128],
                                      selT_d[b, :, TLO * 128:(TLO + TH) * 128])
                    # ar expand per tile
                    arx_ps = ps_pool.tile([128, TPB, H], F32, tag="psA")
                    real_ts = list(range(TL)) + list(range(TLO, TLO + TH))
                    for t in real_ts:
                        nc.tensor.matmul(arx_ps[:, t, :],
                                         selT[:, t * 128:(t + 1) * 128],
                                         ar_sb[:, b, :], start=True, stop=True)
                    # rows are pure bf16 h
                    glo_h = glo[:, :, :].bitcast(mybir.dt.bfloat16)
                    ghi_h = ghi[:, :, :].bitcast(mybir.dt.bfloat16)
                    # al = per-head reduce of h * a_src (row carries no al)
                    alw = wp_pool.tile([128, TPB, F], BF, tag="alw")
                    al_e = wp_pool.tile([128, TPB, H], F32, tag="ale")
                    for r0, r1, gh, T in ((0, TL, glo_h, TL),
                                          (TLO, TLO + TH, ghi_h, TH)):
                        nc.vector.tensor_tensor(
                            alw[:, r0:r1, :].rearrange("p t (h c) -> p t h c", c=C),
                            gh[:, :T, :].rearrange("p t (h c) -> p t h c", c=C),
                            avec_sb[:].rearrange("p (h c) -> p h c", c=C)
                            .unsqueeze(1).broadcast_to([128, T, H, C]),
                            AX.mult)
                        nc.vector.tensor_reduce(
                            al_e[:, r0:r1, :],
                            alw[:, r0:r1, :].rearrange("p t (h c) -> p t h c", c=C),
                            axis=mybir.AxisListType.X, op=AX.add)
                    # e = al + ar ; lrelu (scalar engine); exp -> p
                    wpt = wp_pool.tile([128, TPB, FH], MMDT, tag="wpt")
                    e_sb = wp_pool.tile([128, TPB, H], F32, tag="e")
                    nc.vector.tensor_tensor(e_sb[:, :TL, :],
                                            al_e[:, :TL, :],
                                            arx_ps[:, :TL, :], AX.add)
                    nc.vector.tensor_tensor(e_sb[:, TLO:TLO + TH, :],
                                            al_e[:, TLO:TLO + TH, :],
                                            arx_ps[:, TLO:TLO + TH, :], AX.add)
                    eneg = wp_pool.tile([128, TPB, H], F32, tag="eneg")
                    for r0, r1 in ((0, TL), (TLO, TLO + TH)):
                        nc.vector.tensor_scalar(eneg[:, r0:r1, :], e_sb[:, r0:r1, :],
                                                cfg.neg_slope, None, AX.mult)
                        nc.vector.tensor_tensor(e_sb[:, r0:r1, :], e_sb[:, r0:r1, :],
                                                eneg[:, r0:r1, :], AX.max)
                    nc.scalar.activation(wpt[:, :TL, F:FH], e_sb[:, :TL, :], AF.Exp)
                    nc.scalar.activation(wpt[:, TLO:TLO + TH, F:FH],
                                         e_sb[:, TLO:TLO + TH, :], AF.Exp)
                    nc.vector.tensor_tensor(
                        wpt[:, :TL, :F].rearrange("p t (h c) -> p t h c", c=C),
                        glo_h[:, :TL, :].rearrange("p t (h c) -> p t h c", c=C),
                        wpt[:, :TL, F:FH].unsqueeze(3).broadcast_to([128, TL, H, C]),
                        AX.mult)
                    nc.vector.tensor_tensor(
                        wpt[:, TLO:TLO + TH, :F].rearrange("p t (h c) -> p t h c", c=C),
                        ghi_h[:, :TH, :].rearrange("p t (h c) -> p t h c", c=C),
                        wpt[:, TLO:TLO + TH, F:FH].unsqueeze(3).broadcast_to(
                            [128, TH, H, C]),
                        AX.mult)
                    # scatter matmuls
                    acc_ps = ps_pool.tile([128, FH], F32, tag="psB")
                    for i, t in enumerate(real_ts):
                        nc.tensor.matmul(acc_ps[:], sel[:, t, :], wpt[:, t, :],
                                         start=(i == 0),
                                         stop=(i == len(real_ts) - 1))
                    # divide + bias -> h_cur
                    s_sb = wp_pool.tile([128, H], F32, tag="s")
                    nc.vector.tensor_scalar(s_sb[:], acc_ps[:, F:FH], 1e-30, None,
                                            AX.max)
                    r_sb = wp_pool.tile([128, H], F32, tag="r")
                    nc.vector.reciprocal(r_sb[:], s_sb[:])
                    nc.vector.tensor_tensor(
                        h_cur[:, b, :].rearrange("p (h c) -> p h c", c=C),
                        acc_ps[:, :F].rearrange("p (h c) -> p h c", c=C),
                        r_sb[:].unsqueeze(2).broadcast_to([128, H, C]),
                        AX.mult)
                    nc.vector.tensor_tensor(h_cur[:, b, :], h_cur[:, b, :],
                                            bbc_sb[:], AX.add)
                    # fused BN stats accumulation (sum, sum-of-squares)
                    rhs = wp_pool.tile([128, 2 * F], F32, tag="bnrhs")
                    nc.vector.tensor_scalar(rhs[:, :F], h_cur[:, b, :],
                                            mask_sb[:, b].unsqueeze(1), None, AX.mult)
                    nc.scalar.activation(rhs[:, F:], rhs[:, :F], AF.Square)
                    nc.tensor.matmul(bn_ps[:], onescol[:], rhs[:],
                                     start=(b == 0), stop=(b == NB - 1))
                return bn_ps

            def bn_elu_phase(li, g_sb, be_sb, bn_ps, pool_ps=None):
                bn_sb = wp_pool.tile([1, 2 * F], F32, tag="bnrow")
                nc.vector.tensor_copy(bn_sb[:], bn_ps[:])
                nc.sync.dma_start(bn_in[li][:], bn_sb[:])
                nc.gpsimd.collective_compute(
                    "AllReduce", AX.add, replica_groups=rg,
                    ins=[bn_in[li][:]], outs=[bn_out[li][:]])
                st = wp_pool.tile([1, 2 * F], F32, tag="bnst")
                nc.sync.dma_start(st[:], bn_out[li][:])
                # mu = s/n ; var = ss/n - mu^2
                mu = wp_pool.tile([1, F], F32, tag="mu")
                nc.vector.tensor_scalar(mu[:], st[:, :F], 1.0 / cfg.n_real, None,
                                        AX.mult)
                var = wp_pool.tile([1, F], F32, tag="var")
                nc.vector.tensor_scalar(var[:], st[:, F:], 1.0 / cfg.n_real, None,
                                        AX.mult)
                mu2 = wp_pool.tile([1, F], F32, tag="mu2")
                nc.scalar.activation(mu2[:], mu[:], AF.Square)
                nc.vector.tensor_tensor(var[:], var[:], mu2[:], AX.subtract)
                # rstd = 1/sqrt(var+eps)
                nc.vector.tensor_scalar(var[:], var[:], cfg.eps, None, AX.add)
                sd = wp_pool.tile([1, F], F32, tag="sd")
                nc.scalar.activation(sd[:], var[:], AF.Sqrt)
                rstd = wp_pool.tile([1, F], F32, tag="rstd")
                nc.vector.reciprocal(rstd[:], sd[:])
                # scale = g*rstd ; shift = be - mu*scale
                ssrow = wp_pool.tile([1, 2 * F], F32, tag="ssrow")
                nc.vector.tensor_tensor(ssrow[:, :F], g_sb[:], rstd[:], AX.mult)
                musc = wp_pool.tile([1, F], F32, tag="musc")
                nc.vector.tensor_tensor(musc[:], mu[:], ssrow[:, :F], AX.mult)
                nc.vector.tensor_tensor(ssrow[:, F:], be_sb[:], musc[:], AX.subtract)
                # broadcast via K=1 matmul
                bc_ps = ps1_pool.tile([128, 2 * F], F32, tag="ps1")
                nc.tensor.matmul(bc_ps[:], onesrow[:], ssrow[:], start=True, stop=True)
                bc_sb = wp_pool.tile([128, 2 * F], F32, tag="bnbcsb")
                nc.vector.tensor_copy(bc_sb[:], bc_ps[:])
                # normalize + elu
                for b in range(NB):
                    nc.vector.tensor_tensor(h_cur[:, b, :], h_cur[:, b, :],
                                            bc_sb[:, :F], AX.mult)
                    nc.vector.tensor_tensor(h_cur[:, b, :], h_cur[:, b, :],
                                            bc_sb[:, F:], AX.add)
                    neg = wp_pool.tile([128, F], F32, tag="neg")
                    nc.vector.tensor_scalar(neg[:], h_cur[:, b, :], 0.0, None, AX.min)
                    ex = wp_pool.tile([128, F], F32, tag="ex")
                    nc.scalar.activation(ex[:], neg[:], AF.Exp)
                    nc.vector.tensor_scalar(h_cur[:, b, :], h_cur[:, b, :], 0.0, None,
                                            AX.max)
                    nc.vector.tensor_tensor(h_cur[:, b, :], h_cur[:, b, :], ex[:],
                                            AX.add)
                    nc.vector.tensor_scalar(h_cur[:, b, :], h_cur[:, b, :], -1.0,
                                            None, AX.add)
                    if pool_ps is not None:
                        # fused global-mean-pool accumulation (host gsel, bf16)
                        gsel = wp_pool.tile([128, GP], BF, tag="gsel")
                        nc.sync.dma_start(gsel[:], gselp_d[b, :, :])
                        hbf = wp_pool.tile([128, F], BF, tag="hbf")
                        nc.vector.tensor_copy(hbf[:], h_cur[:, b, :])
                        nc.tensor.matmul(pool_ps[:], hbf[:], gsel[:],
                                         start=(b == 0), stop=(b == NB - 1))

            # ================= program =================
            for _rep in range(cfg.repeat):
              dense_phase(0, w1e_sb)
              bn1_ps = scatter_phase(0, b1bc_sb, avec1_sb)
              bn_elu_phase(0, g1_sb, be1_sb, bn1_ps)
              dense_phase(1, w2e_sb)
              bn2_ps = scatter_phase(1, b2bc_sb, avec2_sb)
              pool_ps = ps1_pool.tile([128, GP], F32, tag="ps2")
              bn_elu_phase(1, g2_sb, be2_sb, bn2_ps, pool_ps=pool_ps)
            # FC head
            try:
                pool_sb = wp_pool.tile([128, GP], F32, tag="poolsb")
                nc.vector.tensor_copy(pool_sb[:], pool_ps[:])
                fc_ps = ps1_pool.tile([cfg.k, GP], F32, tag="ps1")
                nc.tensor.matmul(fc_ps[:], fcw_sb[:], pool_sb[:], start=True, stop=True)
                fc_sb = wp_pool.tile([cfg.k, GP], F32, tag="fcsb")
                nc.vector.tensor_copy(fc_sb[:], fc_ps[:])
                nc.sync.dma_start(fc_in[:], fc_sb[:])
                nc.gpsimd.collective_compute("AllReduce", AX.add, replica_groups=rg,
                                             ins=[fc_in[:]], outs=[fc_out[:]])
                fin = wp_pool.tile([cfg.k, GP], F32, tag="fin")
                nc.sync.dma_start(fin[:], fc_out[:])
                nc.vector.tensor_tensor(fin[:], fin[:], rcnt_sb[:], AX.mult)
                nc.vector.tensor_tensor(fin[:], fin[:], fcbbc_sb[:], AX.add)
                nc.sync.dma_start(out_d[:], fin[:])
            except _SkipRest:
                pass

    nc.compile()
    return nc


# ---------------------------------------------------------------------------
# harness entry point: full inputs in, full output out
# ---------------------------------------------------------------------------

_NC_CACHE = {}


def kernel(**inputs):
    """Full-input GAT forward on 8 NeuronCores. Returns [500, 6] float32."""
    from concourse.bass_utils import run_bass_kernel_spmd

    cfg = Cfg(bf16_rows=True, bf16_mm=True)
    in_maps = prep_inputs(cfg, inputs)
    key = (cfg.tlo, cfg.thi, cfg.bf16_rows, cfg.bf16_mm)
    if key not in _NC_CACHE:
        _NC_CACHE[key] = build_nc(cfg)
    nc = _NC_CACHE[key]
    res = run_bass_kernel_spmd(nc, in_maps, core_ids=list(range(cfg.ncores)))
    out = res.results[0]["out"]
    return np.ascontiguousarray(out[:, :cfg.g].T).astype(np.float32)



# revision 40
# speedup vs baseline: 1.6106x; 1.0144x over previous
"""GAT kernel for TRN2: host prep + Bass program builder + numpy model.

Sharding: nodes (and their in-edges) partitioned across cores by contiguous
shard; per dst-block-of-128 selector-matmul scatter; edge gathers of packed
table rows [h(128f32) | al(4f32) | pad] = 768B via gpsimd dma_gather with a
lo/hi table split (int16 index limit); inter-layer AllGather of the table;
BN via AllReduce of partial sums; pooling via transposed graph-selector
matmul; tiny FC + final AllReduce.
"""
from dataclasses import dataclass

import numpy as np

import concourse.bacc as bacc
import concourse.bass as bass
import concourse.mybir as mybir
import concourse.tile as tile
from concourse import library_config

F32 = mybir.dt.float32
I16 = mybir.dt.int16
I8 = mybir.dt.int8
AX = mybir.AluOpType
AF = mybir.ActivationFunctionType


class _SkipRest(Exception):
    pass



@dataclass
class Cfg:
    ncores: int = 8
    n_real: int = 50000       # real nodes
    np_: int = 50176          # padded nodes (multiple of ncores*128)
    e_raw: int = 800000       # edges before self loops
    g: int = 500              # graphs
    gp: int = 512             # padded graphs (pool matmul free dim)
    tlo: int = 0              # max tiles per block from lo table (computed in prep)
    thi: int = 0
    tlo_b: tuple = ()         # per-block lo tile counts (max over cores)
    thi_b: tuple = ()
    f: int = 128              # features (in = out = 128)
    h: int = 4
    c: int = 32
    k: int = 6
    eps: float = 1e-5
    rowf: int = 192           # table row floats (768B); 128 when bf16_rows
    stage: int = 9            # debug: how much of the program to emit
    repeat: int = 1           # timing: execute the whole body N times
    bf16_rows: bool = False   # pack h as bf16 in 512B table rows
    bf16_mm: bool = False     # bf16 selectors + weighted features (FWL matmuls)
    single_packet: bool = True  # coalesce gather descriptor stream packets
    gather_chunks: int = 2    # split each lo/hi gather into N queue-parallel chunks
    neg_slope: float = 0.2

    @property
    def shard(self):
        return self.np_ // self.ncores

    @property
    def nblk(self):
        return self.shard // 128

    @property
    def arows(self):
        return (self.nblk // 2) * 128      # 3072: shard rows in table A

    @property
    def brows(self):
        return self.shard - self.arows     # 3200: shard rows in table B

    @property
    def tpb(self):
        return self.tlo + self.thi


def fold_attn(a, H, C):
    A = np.zeros((H * C, H), np.float32)
    for h in range(H):
        A[h * C:(h + 1) * C, h] = a[h]
    return A


def pack_idx16(idx):
    """int array [n] (n % 128 == 0) -> [128, n//16] int16 dma_gather layout."""
    n = len(idx)
    arr = np.zeros((16, n // 16), dtype=np.int16)
    arr[np.arange(n) % 16, np.arange(n) // 16] = idx
    return np.tile(arr, (8, 1))


def prep_edges(cfg: Cfg, edge_index):
    """Returns srcp/dstp [ncores, nblk, tpb, 128] (int64; dst sentinel 999)
    and sets cfg.tlo/thi. Srcs are split into table A (shard offset < arows)
    and table B, matching the split AllGather layout."""
    n, npd, sh = cfg.n_real, cfg.np_, cfg.shard
    AR, BR = cfg.arows, cfg.brows
    src = np.concatenate([edge_index[0], np.arange(n)]).astype(np.int64)
    dst = np.concatenate([edge_index[1], np.arange(n)]).astype(np.int64)
    buckets = [[None] * cfg.nblk for _ in range(cfg.ncores)]
    tlo_b = [1] * cfg.nblk
    thi_b = [1] * cfg.nblk
    for ci in range(cfg.ncores):
        m = (dst // sh) == ci
        s, d = src[m], dst[m] - ci * sh
        for b in range(cfg.nblk):
            mb = (d // 128) == b
            sb, db = s[mb], d[mb] % 128
            core_s, off = sb // sh, sb % sh
            lo = off < AR
            rowa = core_s[lo] * AR + off[lo]
            rowb = core_s[~lo] * BR + (off[~lo] - AR)
            buckets[ci][b] = ((rowa, db[lo]), (rowb, db[~lo]))
            tlo_b[b] = max(tlo_b[b], -(-len(rowa) // 128))
            thi_b[b] = max(thi_b[b], -(-len(rowb) // 128))
    cfg.tlo, cfg.thi = max(tlo_b), max(thi_b)
    cfg.tlo_b, cfg.thi_b = tuple(tlo_b), tuple(thi_b)
    tlo, thi = cfg.tlo, cfg.thi
    tpb = cfg.tpb
    srcp = np.zeros((cfg.ncores, cfg.nblk, tpb, 128), np.int64)
    dstp = np.full((cfg.ncores, cfg.nblk, tpb, 128), 999, np.int64)
    for ci in range(cfg.ncores):
        for b in range(cfg.nblk):
            (slo, dlo), (shi, dhi) = buckets[ci][b]
            srcp[ci, b, :tlo].flat[: len(slo)] = slo
            dstp[ci, b, :tlo].flat[: len(dlo)] = dlo
            srcp[ci, b, tlo:].flat[: len(shi)] = shi
            dstp[ci, b, tlo:].flat[: len(dhi)] = dhi
    return srcp, dstp


def prep_inputs(cfg: Cfg, inputs):
    """Build per-core in_maps (list of dicts)."""
    H, C, F = cfg.h, cfg.c, cfg.f
    srcp, dstp = prep_edges(cfg, inputs["edge_index"])
    tlo, thi, tpb = cfg.tlo, cfg.thi, cfg.tpb

    xpad = np.zeros((cfg.np_, F), np.float32)
    xpad[: cfg.n_real] = inputs["x"]

    bf16 = mybir.dt.np(mybir.dt.bfloat16)
    W1e = np.concatenate(
        [inputs["W1"], inputs["W1"] @ fold_attn(inputs["a_dst1"], H, C)], axis=1)
    W2e = np.concatenate(
        [inputs["W2"], inputs["W2"] @ fold_attn(inputs["a_dst2"], H, C)], axis=1)

    batch = np.asarray(inputs["batch"]).astype(np.int64)
    batch_pad = np.full(cfg.np_, 999, np.int64)
    batch_pad[: cfg.n_real] = batch
    cnt = np.bincount(batch, minlength=cfg.gp).astype(np.float32)
    rcnt = (1.0 / np.maximum(cnt, 1.0)).astype(np.float32)

    shared = dict(
        w1e=W1e.astype(np.float32), w2e=W2e.astype(np.float32),
        b1bc=np.tile(inputs["b1"][None, :], (128, 1)).astype(np.float32),
        b2bc=np.tile(inputs["b2"][None, :], (128, 1)).astype(np.float32),
        g1row=inputs["g1"][None, :].astype(np.float32),
        be1row=inputs["be1"][None, :].astype(np.float32),
        g2row=inputs["g2"][None, :].astype(np.float32),
        be2row=inputs["be2"][None, :].astype(np.float32),
        fcw=inputs["fcW"].astype(np.float32),
        fcbbc=np.tile(inputs["fcb"][:, None], (1, cfg.gp)).astype(np.float32),
        rcntbc=np.tile(rcnt[None, :], (cfg.k, 1)).astype(np.float32),
        ident=np.eye(128, dtype=np.float32),
        avec1=np.tile(np.asarray(inputs["a_src1"]).reshape(1, -1),
                      (128, 1)).astype(bf16),
        avec2=np.tile(np.asarray(inputs["a_src2"]).reshape(1, -1),
                      (128, 1)).astype(bf16),
        onescol=np.ones((128, 1), np.float32),
        onesrow=np.ones((1, 128), np.float32),
    )

    in_maps = []
    iota128 = np.arange(128, dtype=np.int64)
    seldt = mybir.dt.np(mybir.dt.float8e4) if cfg.bf16_mm else np.float32
    for ci in range(cfg.ncores):
        idx_lo = np.zeros((cfg.nblk, 128, tlo * 8), np.int16)
        idx_hi = np.zeros((cfg.nblk, 128, thi * 8), np.int16)
        for b in range(cfg.nblk):
            idx_lo[b] = pack_idx16(srcp[ci, b, :tlo].reshape(-1))
            idx_hi[b] = pack_idx16(srcp[ci, b, tlo:].reshape(-1))
        dst_blk = dstp[ci]  # [nblk, tpb, 128] (999 pad)
        # selp[b, p, t, j] = (dst[b, t, p] == j): stationary scatter selector
        selp = (dst_blk.transpose(0, 2, 1)[:, :, :, None]
                == iota128[None, None, None, :]).astype(seldt)
        # selT[b, p, t*128+k] = (dst[b, t, k] == p): arx gather selector
        selT = (dst_blk[:, None, :, :]
                == iota128[None, :, None, None]).astype(seldt)
        selT = selT.reshape(cfg.nblk, 128, tpb * 128)
        sl = slice(ci * cfg.shard, (ci + 1) * cfg.shard)
        nm = np.zeros((128, cfg.nblk), np.float32)
        bc = np.zeros((128, cfg.nblk), np.float32)
        ids = np.arange(ci * cfg.shard, (ci + 1) * cfg.shard)
        nm[:] = (ids.reshape(cfg.nblk, 128).T < cfg.n_real)
        bc[:] = batch_pad[ids].reshape(cfg.nblk, 128).T.astype(np.float32)
        # host-precomputed pool selector [NB, 128, GP] (999 pad -> all-zero)
        gselp = (bc.T[:, :, None]
                 == np.arange(cfg.gp, dtype=np.float32)[None, None, :]).astype(bf16)
        xs = xpad[sl].reshape(cfg.nblk, 128, F).transpose(1, 0, 2)
        in_maps.append(dict(
            x_shard=np.ascontiguousarray(xs).reshape(128, cfg.nblk * F),
            idx_lo=idx_lo, idx_hi=idx_hi,
            selp=selp,
            selT=selT,
            gselp=gselp,
            node_mask=nm, batchcol=bc,
            **shared,
        ))
    return in_maps


# ---------------------------------------------------------------------------
# numpy model (for validation at any cfg)
# ---------------------------------------------------------------------------

def numpy_forward(cfg: Cfg, inputs):
    H, C, F = cfg.h, cfg.c, cfg.f
    srcp, dstp = prep_edges(cfg, inputs["edge_index"])
    xpad = np.zeros((cfg.np_, F), np.float32)
    xpad[: cfg.n_real] = inputs["x"]

    def layer(xp, W, asrc, adst, b):
        We = np.concatenate([W, W @ fold_attn(asrc, H, C), W @ fold_attn(adst, H, C)], 1)
        tab = xp @ We
        out = np.zeros((cfg.np_, F), np.float32)
        for ci in range(cfg.ncores):
            for bi in range(cfg.nblk):
                base = ci * cfg.shard + bi * 128
                acc = np.zeros((128, F + H), np.float32)
                ar_blk = tab[base: base + 128, F + H: F + 2 * H]
                for t in range(cfg.tpb):
                    s = srcp[ci, bi, t] + (cfg.half if t >= cfg.tlo else 0)
                    dl = dstp[ci, bi, t]
                    grow = tab[s]
                    sel = (dl[:, None] == np.arange(128)[None, :]).astype(np.float32)
                    e = grow[:, F:F + H] + sel @ ar_blk
                    e = np.where(e > 0, e, cfg.neg_slope * e).astype(np.float32)
                    p = np.exp(e).astype(np.float32)
                    w = grow[:, :F] * np.repeat(p, C, 1)
                    acc += sel.T @ np.concatenate([w, p], 1)
                ssum = np.maximum(np.repeat(acc[:, F:], C, 1), 1e-30)
                out[base:base + 128] = acc[:, :F] / ssum + b
        return out

    def bn_elu(hh, g, be):
        s, ss = hh[:cfg.n_real].sum(0), (hh[:cfg.n_real] ** 2).sum(0)
        mu = s / cfg.n_real
        var = ss / cfg.n_real - mu ** 2
        sc = g / np.sqrt(var + cfg.eps)
        sh = be - mu * sc
        y = hh * sc + sh
        return (np.where(y > 0, y, np.exp(np.minimum(y, 0)) - 1)).astype(np.float32)

    h1 = layer(xpad, inputs["W1"], inputs["a_src1"], inputs["a_dst1"], inputs["b1"])
    h1n = bn_elu(h1, inputs["g1"], inputs["be1"])
    h2 = layer(h1n, inputs["W2"], inputs["a_src2"], inputs["a_dst2"], inputs["b2"])
    h2n = bn_elu(h2, inputs["g2"], inputs["be2"])

    batch = np.asarray(inputs["batch"]).astype(np.int64)
    gsel = np.zeros((cfg.n_real, cfg.gp), np.float32)
    gsel[np.arange(cfg.n_real), batch] = 1.0
    pooled = h2n[:cfg.n_real].T @ gsel
    fc = inputs["fcW"].T.astype(np.float32) @ pooled
    cnt = np.bincount(batch, minlength=cfg.gp).astype(np.float32)
    fc = fc / np.maximum(cnt, 1.0)[None, :] + inputs["fcb"][:, None]
    return fc[:, :cfg.g].T  # [g, k]


# ---------------------------------------------------------------------------
# Bass program
# ---------------------------------------------------------------------------

def build_nc(cfg: Cfg):
    NB, TPB, TLO, THI = cfg.nblk, cfg.tpb, cfg.tlo, cfg.thi
    F, H, C, RF = cfg.f, cfg.h, cfg.c, cfg.rowf
    FH = F + H
    SH = cfg.shard
    GP = cfg.gp

    if cfg.bf16_mm:
        assert cfg.bf16_rows, "bf16_mm requires bf16_rows"
    if cfg.bf16_rows:
        cfg.rowf = 64              # 256B rows: h in bf16 only
        RF = 64
    BF = mybir.dt.bfloat16
    MMDT = BF if cfg.bf16_mm else F32
    SELDT = mybir.dt.float8e4 if cfg.bf16_mm else F32
    nc = bacc.Bacc("TRN2", target_bir_lowering=False, debug=False,
                   num_devices=cfg.ncores, num_swdge_queues=4)

    def ext(name, shape, dtype=F32):
        return nc.dram_tensor(name, shape, dtype, kind="ExternalInput")

    x_shard = ext("x_shard", [128, NB * F])
    idx_lo = ext("idx_lo", [NB, 128, TLO * 8], I16)
    idx_hi = ext("idx_hi", [NB, 128, THI * 8], I16)
    selp_d = ext("selp", [NB, 128, TPB, 128], SELDT)
    selT_d = ext("selT", [NB, 128, TPB * 128], SELDT)
    node_mask = ext("node_mask", [128, NB])
    batchcol = ext("batchcol", [128, NB])
    gselp_d = ext("gselp", [NB, 128, GP], BF)
    w1e = ext("w1e", [F, F + H])
    w2e = ext("w2e", [F, F + H])
    avec1_d = ext("avec1", [128, F], BF)
    avec2_d = ext("avec2", [128, F], BF)
    b1bc = ext("b1bc", [128, F])
    b2bc = ext("b2bc", [128, F])
    g1row = ext("g1row", [1, F])
    be1row = ext("be1row", [1, F])
    g2row = ext("g2row", [1, F])
    be2row = ext("be2row", [1, F])
    fcw = ext("fcw", [F, cfg.k])
    fcbbc = ext("fcbbc", [cfg.k, GP])
    rcntbc = ext("rcntbc", [cfg.k, GP])
    ident_d = ext("ident", [128, 128])
    onescol_d = ext("onescol", [128, 1])
    onesrow_d = ext("onesrow", [1, 128])

    out_d = nc.dram_tensor("out", [cfg.k, GP], F32, kind="ExternalOutput")

    rg = [list(range(cfg.ncores))]
    shared_as = "Shared" if cfg.ncores > 4 else "Local"

    with tile.TileContext(nc) as tc:
        with (
            tc.tile_pool(name="dram", bufs=1, space="DRAM") as dpool,
            tc.tile_pool(name="persist", bufs=1) as pp,
            tc.tile_pool(name="consts", bufs=1) as cp,
            tc.tile_pool(name="work", bufs=3) as wp_pool,
            tc.tile_pool(name="gath", bufs=4) as gp_pool,
            tc.tile_pool(name="psum", bufs=3, space="PSUM") as ps_pool,
            tc.tile_pool(name="psum1", bufs=1, space="PSUM") as ps1_pool,
        ):
            nc.gpsimd.load_library(library_config.mlp)

            # ---- persistent SBUF ----
            h_cur = pp.tile([128, NB, F], F32)          # shard activations
            ar_sb = pp.tile([128, NB, H], MMDT)
            mask_sb = pp.tile([128, NB], F32)
            bcol_sb = pp.tile([128, NB], F32)

            # ---- constants ----
            w1e_sb = cp.tile([128, F + H], F32)
            w2e_sb = cp.tile([128, F + H], F32)
            b1bc_sb = cp.tile([128, F], F32)
            b2bc_sb = cp.tile([128, F], F32)
            ident = cp.tile([128, 128], F32)
            avec1_sb = cp.tile([128, F], BF)
            avec2_sb = cp.tile([128, F], BF)
            onescol = cp.tile([128, 1], F32)
            onesrow = cp.tile([1, 128], F32)
            g1_sb = cp.tile([1, F], F32)
            be1_sb = cp.tile([1, F], F32)
            g2_sb = cp.tile([1, F], F32)
            be2_sb = cp.tile([1, F], F32)
            fcw_sb = cp.tile([128, cfg.k], F32)
            fcbbc_sb = cp.tile([cfg.k, GP], F32)
            rcnt_sb = cp.tile([cfg.k, GP], F32)

            for sb, d in [(w1e_sb, w1e), (w2e_sb, w2e), (b1bc_sb, b1bc),
                          (b2bc_sb, b2bc), (ident, ident_d),
                          (avec1_sb, avec1_d), (avec2_sb, avec2_d),
                          (onescol, onescol_d),
                          (onesrow, onesrow_d), (g1_sb, g1row), (be1_sb, be1row),
                          (g2_sb, g2row), (be2_sb, be2row), (fcw_sb, fcw),
                          (fcbbc_sb, fcbbc), (rcnt_sb, rcntbc),
                          (mask_sb, node_mask), (bcol_sb, batchcol),
                          (h_cur, x_shard)]:
                nc.sync.dma_start(sb[:], d[:])

            # ---- DRAM internals ----
            AR, BR = cfg.arows, cfg.brows
            ht_in = [dpool.tile([SH, RF], F32, name=f"ht{i}_in") for i in (1, 2)]
            ht_a = [dpool.tile([cfg.ncores * AR, RF], F32, addr_space=shared_as,
                               name=f"ht{i}a") for i in (1, 2)]
            ht_b = [dpool.tile([cfg.ncores * BR, RF], F32, addr_space=shared_as,
                               name=f"ht{i}b") for i in (1, 2)]
            bn_in = [dpool.tile([1, 2 * F], F32, name=f"bn{i}_in") for i in (1, 2)]
            bn_out = [dpool.tile([1, 2 * F], F32, addr_space=shared_as, name=f"bn{i}_out")
                      for i in (1, 2)]
            fc_in = dpool.tile([cfg.k, GP], F32)
            fc_out = dpool.tile([cfg.k, GP], F32, addr_space=shared_as)

            # ================= helper phases =================

            def norm_elu_block(b, bc_sb):
                """in-place BN-normalize + ELU of h_cur[:, b, :]."""
                nc.vector.tensor_tensor(h_cur[:, b, :], h_cur[:, b, :],
                                        bc_sb[:, :F], AX.mult)
                nc.vector.tensor_tensor(h_cur[:, b, :], h_cur[:, b, :],
                                        bc_sb[:, F:], AX.add)
                neg = wp_pool.tile([128, F], F32, tag="neg")
                nc.vector.tensor_scalar(neg[:], h_cur[:, b, :], 0.0, None, AX.min)
                ex = wp_pool.tile([128, F], F32, tag="ex")
                nc.scalar.activation(ex[:], neg[:], AF.Exp)
                nc.vector.tensor_scalar(h_cur[:, b, :], h_cur[:, b, :], 0.0, None,
                                        AX.max)
                nc.vector.tensor_tensor(h_cur[:, b, :], h_cur[:, b, :], ex[:],
                                        AX.add)
                nc.vector.tensor_scalar(h_cur[:, b, :], h_cur[:, b, :], -1.0,
                                        None, AX.add)

            def dense_phase(li, wext_sb, bc_sb=None):
                """h_cur -> table rows (HT_in) + ar_sb; split AllGather A/B.
                With bc_sb, fuses the BN-normalize+ELU of each block inline."""
                nba = AR // 128
                for b in range(NB):
                    if bc_sb is not None:
                        norm_elu_block(b, bc_sb)
                    tr_ps = ps_pool.tile([128, 128], F32, tag="psA")
                    nc.tensor.transpose(tr_ps[:], h_cur[:, b, :], ident[:])
                    xT = wp_pool.tile([128, 128], F32, tag="xT")
                    nc.scalar.activation(xT[:], tr_ps[:], AF.Copy)
                    dp_ps = ps_pool.tile([128, F + H], F32, tag="psB")
                    nc.tensor.matmul(dp_ps[:], xT[:], wext_sb[:], start=True, stop=True)
                    row = wp_pool.tile([128, RF], F32, tag="row")
                    nc.scalar.activation(
                        row[:, :RF].bitcast(mybir.dt.bfloat16), dp_ps[:, :F],
                        AF.Copy)
                    nc.vector.tensor_copy(ar_sb[:, b, :], dp_ps[:, F:F + H])
                    nc.sync.dma_start(
                        ht_in[li][b * 128:(b + 1) * 128, :],
                        row[:])
                    if b == nba - 1:
                        nc.gpsimd.collective_compute(
                            "AllGather", AX.bypass, replica_groups=rg,
                            ins=[ht_in[li][:AR, :]], outs=[ht_a[li][:]])
                nc.gpsimd.collective_compute(
                    "AllGather", AX.bypass, replica_groups=rg,
                    ins=[ht_in[li][AR:, :]], outs=[ht_b[li][:]])

            def scatter_phase(li, bbc_sb, avec_sb):
                """edge phase: gathers + selector matmuls -> h_cur (+bias).
                Returns the accumulated BN stats PSUM tile."""
                bn_ps = ps1_pool.tile([1, 2 * F], F32, tag="ps1")
                for b in range(NB):
                    TL, TH = cfg.tlo_b[b], cfg.thi_b[b]
                    ilo_t = gp_pool.tile([128, TLO * 8], I16, tag="ilo")
                    ihi_t = gp_pool.tile([128, THI * 8], I16, tag="ihi")
                    nc.sync.dma_start(ilo_t[:, :TL * 8], idx_lo[b, :, :TL * 8])
                    nc.sync.dma_start(ihi_t[:, :TH * 8], idx_hi[b, :, :TH * 8])
                    glo = gp_pool.tile([128, TLO, RF], F32, tag="glo")
                    ghi = gp_pool.tile([128, THI, RF], F32, tag="ghi")
                    # chunked queue-parallel gathers: lo on queues 0..1, hi 2..3
                    def gather_chunks(dst_t, src_ap, idxs_t, T, qbase):
                        nch = min(cfg.gather_chunks, T)
                        bounds = [T * i // nch for i in range(nch + 1)]
                        for c in range(nch):
                            a, z = bounds[c], bounds[c + 1]
                            if z <= a:
                                continue
                            nc.gpsimd.dma_gather(
                                out_ap=dst_t[:, a:z, :], in_ap=src_ap,
                                idxs_ap=idxs_t[:, a * 8:z * 8],
                                num_idxs=(z - a) * 128,
                                num_idxs_reg=(z - a) * 128, elem_size=RF,
                                queue_num=qbase + c,
                                single_packet=cfg.single_packet)
                    gather_chunks(glo, ht_a[li][:], ilo_t, TL, 0)
                    gather_chunks(ghi, ht_b[li][:], ihi_t, TH, 2)
                    # selectors (host-precomputed, DMA-streamed, fp8)
                    sel = gp_pool.tile([128, TPB, 128], SELDT, tag="sel")
                    selT = gp_pool.tile([128, TPB * 128], SELDT, tag="selT")
                    nc.sync.dma_start(sel[:, :TL, :], selp_d[b, :, :TL, :])
                    nc.sync.dma_start(sel[:, TLO:TLO + TH, :],
                                      selp_d[b, :, TLO:TLO + TH, :])
                    nc.sync.dma_start(selT[:, :TL * 128], selT_d[b, :, :TL * 128])
                    nc.sync.dma_start(selT[:, TLO * 128:(TLO + TH) * 128],
                                      selT_d[b, :, TLO * 128:(TLO + TH) * 128])
                    # ar expand per tile
                    arx_ps = ps_pool.tile([128, TPB, H], F32, tag="psA")
                    real_ts = list(range(TL)) + list(range(TLO, TLO + TH))
                    for t in real_ts:
                        nc.tensor.matmul(arx_ps[:, t, :],
                                         selT[:, t * 128:(t + 1) * 128],
                                         ar_sb[:, b, :], start=True, stop=True)
                    # rows are pure bf16 h
                    glo_h = glo[:, :, :].bitcast(mybir.dt.bfloat16)
                    ghi_h = ghi[:, :, :].bitcast(mybir.dt.bfloat16)
                    # al = per-head reduce of h * a_src (row carries no al)
                    alw = wp_pool.tile([128, TPB, F], BF, tag="alw")
                    al_e = wp_pool.tile([128, TPB, H], F32, tag="ale")
                    for r0, r1, gh, T in ((0, TL, glo_h, TL),
                                          (TLO, TLO + TH, ghi_h, TH)):
                        nc.vector.tensor_tensor(
                            alw[:, r0:r1, :].rearrange("p t (h c) -> p t h c", c=C),
                            gh[:, :T, :].rearrange("p t (h c) -> p t h c", c=C),
                            avec_sb[:].rearrange("p (h c) -> p h c", c=C)
                            .unsqueeze(1).broadcast_to([128, T, H, C]),
                            AX.mult)
                        nc.vector.tensor_reduce(
                            al_e[:, r0:r1, :],
                            alw[:, r0:r1, :].rearrange("p t (h c) -> p t h c", c=C),
                            axis=mybir.AxisListType.X, op=AX.add)
                    # e = al + ar ; lrelu (scalar engine); exp -> p
                    wpt = wp_pool.tile([128, TPB, FH], MMDT, tag="wpt")
                    e_sb = wp_pool.tile([128, TPB, H], F32, tag="e")
                    nc.vector.tensor_tensor(e_sb[:, :TL, :],
                                            al_e[:, :TL, :],
                                            arx_ps[:, :TL, :], AX.add)
                    nc.vector.tensor_tensor(e_sb[:, TLO:TLO + TH, :],
                                            al_e[:, TLO:TLO + TH, :],
                                            arx_ps[:, TLO:TLO + TH, :], AX.add)
                    eneg = wp_pool.tile([128, TPB, H], F32, tag="eneg")
                    for r0, r1 in ((0, TL), (TLO, TLO + TH)):
                        nc.vector.tensor_scalar(eneg[:, r0:r1, :], e_sb[:, r0:r1, :],
                                                cfg.neg_slope, None, AX.mult)
                        nc.vector.tensor_tensor(e_sb[:, r0:r1, :], e_sb[:, r0:r1, :],
                                                eneg[:, r0:r1, :], AX.max)
                    nc.scalar.activation(wpt[:, :TL, F:FH], e_sb[:, :TL, :], AF.Exp)
                    nc.scalar.activation(wpt[:, TLO:TLO + TH, F:FH],
                                         e_sb[:, TLO:TLO + TH, :], AF.Exp)
                    nc.vector.tensor_tensor(
                        wpt[:, :TL, :F].rearrange("p t (h c) -> p t h c", c=C),
                        glo_h[:, :TL, :].rearrange("p t (h c) -> p t h c", c=C),
                        wpt[:, :TL, F:FH].unsqueeze(3).broadcast_to([128, TL, H, C]),
                        AX.mult)
                    nc.vector.tensor_tensor(
                        wpt[:, TLO:TLO + TH, :F].rearrange("p t (h c) -> p t h c", c=C),
                        ghi_h[:, :TH, :].rearrange("p t (h c) -> p t h c", c=C),
                        wpt[:, TLO:TLO + TH, F:FH].unsqueeze(3).broadcast_to(
                            [128, TH, H, C]),
                        AX.mult)
                    # scatter matmuls
                    acc_ps = ps_pool.tile([128, FH], F32, tag="psB")
                    for i, t in enumerate(real_ts):
                        nc.tensor.matmul(acc_ps[:], sel[:, t, :], wpt[:, t, :],
                                         start=(i == 0),
                                         stop=(i == len(real_ts) - 1))
                    # divide + bias -> h_cur
                    s_sb = wp_pool.tile([128, H], F32, tag="s")
                    nc.vector.tensor_scalar(s_sb[:], acc_ps[:, F:FH], 1e-30, None,
                                            AX.max)
                    r_sb = wp_pool.tile([128, H], F32, tag="r")
                    nc.vector.reciprocal(r_sb[:], s_sb[:])
                    nc.vector.tensor_tensor(
                        h_cur[:, b, :].rearrange("p (h c) -> p h c", c=C),
                        acc_ps[:, :F].rearrange("p (h c) -> p h c", c=C),
                        r_sb[:].unsqueeze(2).broadcast_to([128, H, C]),
                        AX.mult)
                    nc.vector.tensor_tensor(h_cur[:, b, :], h_cur[:, b, :],
                                            bbc_sb[:], AX.add)
                    # fused BN stats accumulation (sum, sum-of-squares)
                    rhs = wp_pool.tile([128, 2 * F], F32, tag="bnrhs")
                    nc.vector.tensor_scalar(rhs[:, :F], h_cur[:, b, :],
                                            mask_sb[:, b].unsqueeze(1), None, AX.mult)
                    nc.scalar.activation(rhs[:, F:], rhs[:, :F], AF.Square)
                    nc.tensor.matmul(bn_ps[:], onescol[:], rhs[:],
                                     start=(b == 0), stop=(b == NB - 1))
                return bn_ps

            def bn_elu_phase(li, g_sb, be_sb, bn_ps, pool_ps=None):
                bn_sb = wp_pool.tile([1, 2 * F], F32, tag="bnrow")
                nc.vector.tensor_copy(bn_sb[:], bn_ps[:])
                nc.sync.dma_start(bn_in[li][:], bn_sb[:])
                nc.gpsimd.collective_compute(
                    "AllReduce", AX.add, replica_groups=rg,
                    ins=[bn_in[li][:]], outs=[bn_out[li][:]])
                st = wp_pool.tile([1, 2 * F], F32, tag="bnst")
                nc.sync.dma_start(st[:], bn_out[li][:])
                # mu = s/n ; var = ss/n - mu^2
                mu = wp_pool.tile([1, F], F32, tag="mu")
                nc.vector.tensor_scalar(mu[:], st[:, :F], 1.0 / cfg.n_real, None,
                                        AX.mult)
                var = wp_pool.tile([1, F], F32, tag="var")
                nc.vector.tensor_scalar(var[:], st[:, F:], 1.0 / cfg.n_real, None,
                                        AX.mult)
                mu2 = wp_pool.tile([1, F], F32, tag="mu2")
                nc.scalar.activation(mu2[:], mu[:], AF.Square)
                nc.vector.tensor_tensor(var[:], var[:], mu2[:], AX.subtract)
                # rstd = 1/sqrt(var+eps)
                nc.vector.tensor_scalar(var[:], var[:], cfg.eps, None, AX.add)
                sd = wp_pool.tile([1, F], F32, tag="sd")
                nc.scalar.activation(sd[:], var[:], AF.Sqrt)
                rstd = wp_pool.tile([1, F], F32, tag="rstd")
                nc.vector.reciprocal(rstd[:], sd[:])
                # scale = g*rstd ; shift = be - mu*scale
                ssrow = wp_pool.tile([1, 2 * F], F32, tag="ssrow")
                nc.vector.tensor_tensor(ssrow[:, :F], g_sb[:], rstd[:], AX.mult)
                musc = wp_pool.tile([1, F], F32, tag="musc")
                nc.vector.tensor_tensor(musc[:], mu[:], ssrow[:, :F], AX.mult)
                nc.vector.tensor_tensor(ssrow[:, F:], be_sb[:], musc[:], AX.subtract)
                # broadcast via K=1 matmul
                bc_ps = ps1_pool.tile([128, 2 * F], F32, tag="ps1")
                nc.tensor.matmul(bc_ps[:], onesrow[:], ssrow[:], start=True, stop=True)
                bc_sb = wp_pool.tile([128, 2 * F], F32, tag="bnbcsb")
                nc.vector.tensor_copy(bc_sb[:], bc_ps[:])
                if pool_ps is None:
                    return bc_sb  # caller fuses normalize+elu into next phase
                # normalize + elu + fused global-mean-pool
                for b in range(NB):
                    norm_elu_block(b, bc_sb)
                    gsel = wp_pool.tile([128, GP], BF, tag="gsel")
                    nc.sync.dma_start(gsel[:], gselp_d[b, :, :])
                    hbf = wp_pool.tile([128, F], BF, tag="hbf")
                    nc.vector.tensor_copy(hbf[:], h_cur[:, b, :])
                    nc.tensor.matmul(pool_ps[:], hbf[:], gsel[:],
                                     start=(b == 0), stop=(b == NB - 1))
                return bc_sb

            # ================= program =================
            for _rep in range(cfg.repeat):
              dense_phase(0, w1e_sb)
              bn1_ps = scatter_phase(0, b1bc_sb, avec1_sb)
              bc1_sb = bn_elu_phase(0, g1_sb, be1_sb, bn1_ps)
              dense_phase(1, w2e_sb, bc_sb=bc1_sb)
              bn2_ps = scatter_phase(1, b2bc_sb, avec2_sb)
              pool_ps = ps1_pool.tile([128, GP], F32, tag="ps2")
              bn_elu_phase(1, g2_sb, be2_sb, bn2_ps, pool_ps=pool_ps)
            # FC head
            try:
                pool_sb = wp_pool.tile([128, GP], F32, tag="poolsb")
                nc.vector.tensor_copy(pool_sb[:], pool_ps[:])
                fc_ps = ps1_pool.tile([cfg.k, GP], F32, tag="ps1")
                nc.tensor.matmul(fc_ps[:], fcw_sb[:], pool_sb[:], start=True, stop=True)
                fc_sb = wp_pool.tile([cfg.k, GP], F32, tag="fcsb")
                nc.vector.tensor_copy(fc_sb[:], fc_ps[:])
                nc.sync.dma_start(fc_in[:], fc_sb[:])
                nc.gpsimd.collective_compute("AllReduce", AX.add, replica_groups=rg,
                                             ins=[fc_in[:]], outs=[fc_out[:]])
                fin = wp_pool.tile([cfg.k, GP], F32, tag="fin")
                nc.sync.dma_start(fin[:], fc_out[:])
                nc.vector.tensor_tensor(fin[:], fin[:], rcnt_sb[:], AX.mult)
                nc.vector.tensor_tensor(fin[:], fin[:], fcbbc_sb[:], AX.add)
                nc.sync.dma_start(out_d[:], fin[:])
            except _SkipRest:
                pass

    nc.compile()
    return nc


# ---------------------------------------------------------------------------
# harness entry point: full inputs in, full output out
# ---------------------------------------------------------------------------

_NC_CACHE = {}


def kernel(**inputs):
    """Full-input GAT forward on 8 NeuronCores. Returns [500, 6] float32."""
    from concourse.bass_utils import run_bass_kernel_spmd

    cfg = Cfg(bf16_rows=True, bf16_mm=True)
    in_maps = prep_inputs(cfg, inputs)
    key = (cfg.tlo, cfg.thi, cfg.bf16_rows, cfg.bf16_mm)
    if key not in _NC_CACHE:
        _NC_CACHE[key] = build_nc(cfg)
    nc = _NC_CACHE[key]
    res = run_bass_kernel_spmd(nc, in_maps, core_ids=list(range(cfg.ncores)))
    out = res.results[0]["out"]
    return np.ascontiguousarray(out[:, :cfg.g].T).astype(np.float32)

